# revision 11
# baseline (speedup 1.0000x reference)
"""Trainium2 Bass kernel for BatchedModelManifoldGeodesicFlow.

Math (per sample, derived analytically from the reference's autodiff):
  f(x) = tanh(x@W1 + b1)@W2 + b2,  J = jacrev(f)(x) = W2^T diag(d) W1^T
  with h = tanh(x@W1+b1), d = 1-h^2, e = -2*h*d.
  G = J J^T + eps*I = W2^T diag(d) K diag(d) W2 + eps*I,  K = W1^T W1.
  With L = W2 W2^T, T = diag(d) L diag(d) K, U = K T, Th = diag(h) T:
    ||dG||^2 = 2*( e^T (K.*L.*U) e  +  4 * sum(K .* Th .* Th^T) )
  The Christoffel contraction reduces to small matvecs:
    S1 = W2^T [ e.*cv.*w + d.*(K (e.*g.*w)) ],  S2 = 2 W1 (e.*g.*cv)
    w = W1^T v, g = W2 v, cv = K (d.*g)
    a = (0.5*S2 - S1) / ((||dG||+1e-6) * (||v||+1e-6))
  out = concat([v, a - 0.1*dev], axis=0)

Sharding: pure data parallel, batch 16 -> 2 samples per core on 8 cores.
"""

import sys

if "/opt/trn_rl_repo" not in sys.path:
    sys.path.insert(0, "/opt/trn_rl_repo")

import numpy as np

import concourse.bass as bass
import concourse.bacc as bacc
import concourse.tile as tile
from concourse import mybir
from concourse.masks import make_identity

N = 128
H = 256
B = 16
NCORES = 8
BLOC = B // NCORES  # 2 samples per core

F32 = mybir.dt.float32
I32 = mybir.dt.int32
AF = mybir.ActivationFunctionType
OP = mybir.AluOpType
AX = mybir.AxisListType

# fast fp32 matmul mode (float32r): 4x PE throughput on big matmuls
FAST_MM = False

SQRT_MAGIC = 0x1FBD1DF5  # bits trick: sqrt(x) ~ bitcast((bits(x)>>1) + MAGIC)


def ts(i, sz=128):
    return slice(i * sz, (i + 1) * sz)


def _mm_dt(ap):
    return ap.bitcast(mybir.dt.float32r) if FAST_MM else ap


def build_nc(stage=99):
    """stage: truncate the program for bisection. 99 = full kernel."""
    nc = bacc.Bacc(trn_type="TRN2", enable_partition_id=False)

    d_x0 = nc.dram_tensor("x0", [BLOC, N], F32, kind="ExternalInput")
    d_x1 = nc.dram_tensor("x1", [BLOC, N], F32, kind="ExternalInput")
    d_dev = nc.dram_tensor("dev", [BLOC, N], F32, kind="ExternalInput")
    d_vel = nc.dram_tensor("vel", [BLOC, N], F32, kind="ExternalInput")
    d_w1 = nc.dram_tensor("W1", [N, H], F32, kind="ExternalInput")
    d_w2 = nc.dram_tensor("W2", [H, N], F32, kind="ExternalInput")
    d_b1 = nc.dram_tensor("b1", [1, H], F32, kind="ExternalInput")
    d_t = nc.dram_tensor("t", [1, 1], F32, kind="ExternalInput")
    d_out = nc.dram_tensor("out_bot", [BLOC, N], F32, kind="ExternalOutput")

    with tile.TileContext(nc) as tc:
        with (
            tc.tile_pool(name="consts", bufs=1) as consts,
            tc.tile_pool(name="work", bufs=1) as work,
            tc.tile_pool(name="loop", bufs=2) as loop,
            tc.tile_pool(name="pbig", bufs=4, space="PSUM") as pbig,
            tc.tile_pool(name="psmall", bufs=3, space="PSUM") as psmall,
            tc.tile_pool(name="prow", bufs=1, space="PSUM") as prow,
        ):
            _emit(nc, stage, consts, work, loop, pbig, psmall, prow,
                  d_x0, d_x1, d_dev, d_vel, d_w1, d_w2, d_b1, d_t, d_out)

    nc.compile()  # Bacc: split multi-waits into event semaphores, alloc regs
    return nc


def _emit(nc, stage, consts, work, loop, pbig, psmall, prow,
          d_x0, d_x1, d_dev, d_vel, d_w1, d_w2, d_b1, d_t, d_out):
    # ---------------- input DMA ----------------
    sb_x0 = consts.tile([BLOC, N], F32)
    sb_x1 = consts.tile([BLOC, N], F32)
    sb_dev = consts.tile([BLOC, N], F32)
    sb_vel = consts.tile([BLOC, N], F32)
    sb_w1 = consts.tile([128, H], F32)          # W1[k, m]
    sb_w2 = consts.tile([128, 2, 128], F32)     # W2 tiles: [m%128, m//128, j]
    sb_b1row = consts.tile([1, H], F32)
    sb_t = consts.tile([1, 1], F32)
    nc.sync.dma_start(out=sb_x0, in_=d_x0[:, :])
    nc.sync.dma_start(out=sb_x1, in_=d_x1[:, :])
    nc.sync.dma_start(out=sb_dev, in_=d_dev[:, :])
    nc.sync.dma_start(out=sb_vel, in_=d_vel[:, :])
    nc.sync.dma_start(out=sb_w1, in_=d_w1[:, :])
    for t in range(2):
        nc.sync.dma_start(out=sb_w2[:, t, :], in_=d_w2[ts(t), :])
    nc.sync.dma_start(out=sb_b1row, in_=d_b1[:, :])
    nc.sync.dma_start(out=sb_t, in_=d_t[:, :])

    def debug_out():
        dbg = work.tile([BLOC, N], F32)
        nc.scalar.copy(out=dbg, in_=sb_vel)
        nc.sync.dma_start(out=d_out[:, :], in_=dbg)

    if stage < 1:
        debug_out()
        return

    # ---------------- constants ----------------
    ident = consts.tile([128, 128], F32)
    make_identity(nc, ident)
    ones_row = consts.tile([1, 128], F32)
    nc.vector.memset(ones_row, 1.0)
    ones_col = consts.tile([128, 1], F32)
    nc.vector.memset(ones_col, 1.0)

    # W2^T [j, m] and W1^T blocks [m, i]
    sb_w2t = consts.tile([128, H], F32)
    p_w2t = pbig.tile([128, 256], F32, tag="big")
    for t in range(2):
        nc.tensor.transpose(out=p_w2t[:, ts(t)], in_=sb_w2[:, t, :], identity=ident)
    nc.scalar.copy(out=sb_w2t, in_=p_w2t)

    sb_w1t = consts.tile([128, H], F32)  # block t = (W1[:, t*128:+128])^T
    p_w1t = pbig.tile([128, 256], F32, tag="big")
    for t in range(2):
        nc.tensor.transpose(out=p_w1t[:, ts(t)], in_=sb_w1[:, ts(t)], identity=ident)
    nc.scalar.copy(out=sb_w1t, in_=p_w1t)

    # b1 in column form [m%128, m//128]
    sb_b1c = consts.tile([128, 2], F32)
    p_b1c = psmall.tile([128, 2], F32, tag="small")
    for t in range(2):
        nc.tensor.transpose(
            out=p_b1c[:, t : t + 1], in_=sb_b1row[0:1, ts(t)], identity=ident[0:1, 0:1]
        )
    nc.scalar.copy(out=sb_b1c, in_=p_b1c)

    # K = W1^T W1, L = W2 W2^T, KL = K.*L   (all [128, tile, 256])
    sb_K = consts.tile([128, 2, H], F32)
    sb_L = consts.tile([128, 2, H], F32)
    sb_KL = consts.tile([128, 2, H], F32)
    for t in range(2):
        p_k = pbig.tile([128, 256], F32, tag="big")
        nc.tensor.matmul(
            p_k, _mm_dt(sb_w1[:, ts(t)]), _mm_dt(sb_w1), start=True, stop=True
        )
        nc.scalar.copy(out=sb_K[:, t, :], in_=p_k)
    for t in range(2):
        p_l = pbig.tile([128, 256], F32, tag="big")
        nc.tensor.matmul(
            p_l, _mm_dt(sb_w2t[:, ts(t)]), _mm_dt(sb_w2t), start=True, stop=True
        )
        nc.scalar.copy(out=sb_L[:, t, :], in_=p_l)
    for t in range(2):
        nc.vector.tensor_mul(sb_KL[:, t, :], sb_K[:, t, :], sb_L[:, t, :])

    if stage < 2:
        debug_out()
        return

    # ---------------- t scalar / window ----------------
    p_t128 = psmall.tile([128, 1], F32, tag="small")
    nc.tensor.matmul(p_t128, ones_row[0:1, :], sb_t[0:1, 0:1], start=True, stop=True)
    t128 = work.tile([128, 1], F32)
    nc.scalar.copy(out=t128, in_=p_t128)
    omt = work.tile([128, 1], F32)
    nc.vector.tensor_scalar(
        out=omt, in0=p_t128, scalar1=-1.0, scalar2=1.0, op0=OP.mult, op1=OP.add
    )
    wf = work.tile([128, 1], F32)
    nc.vector.tensor_mul(wf, omt, t128)  # t*(1-t); x4 folded below

    # ---------------- x path: u = x@W1 + b1 without forming x ----------
    dxr = work.tile([BLOC, N], F32)
    nc.vector.tensor_sub(dxr, sb_x1, sb_x0)
    p_x0t = psmall.tile([128, BLOC], F32, tag="small")
    nc.tensor.transpose(out=p_x0t, in_=sb_x0, identity=ident[0:BLOC, 0:BLOC])
    p_dxt = psmall.tile([128, BLOC], F32, tag="small")
    nc.tensor.transpose(out=p_dxt, in_=dxr, identity=ident[0:BLOC, 0:BLOC])
    p_devt = psmall.tile([128, BLOC], F32, tag="small")
    nc.tensor.transpose(out=p_devt, in_=sb_dev, identity=ident[0:BLOC, 0:BLOC])
    p_velt = psmall.tile([128, BLOC], F32, tag="small")
    nc.tensor.transpose(out=p_velt, in_=sb_vel, identity=ident[0:BLOC, 0:BLOC])

    x0t = work.tile([128, BLOC], F32)
    nc.scalar.copy(out=x0t, in_=p_x0t)
    dxt = work.tile([128, BLOC], F32)
    nc.vector.tensor_scalar_mul(dxt, p_dxt, t128)           # t*(x1-x0)^T
    devt = work.tile([128, BLOC], F32)
    nc.vector.tensor_scalar(
        out=devt, in0=p_devt, scalar1=wf, scalar2=4.0, op0=OP.mult, op1=OP.mult
    )                                                        # 4t(1-t)*dev^T
    velt = work.tile([128, BLOC], F32)
    nc.scalar.copy(out=velt, in_=p_velt)

    # u columns per H-tile: [m%128, tile, sample]
    p_uc = psmall.tile([128, 2, BLOC], F32, tag="small")
    for t in range(2):
        nc.tensor.matmul(p_uc[:, t, :], sb_w1[:, ts(t)], x0t, start=True, stop=False)
        nc.tensor.matmul(p_uc[:, t, :], sb_w1[:, ts(t)], dxt, start=False, stop=False)
        nc.tensor.matmul(p_uc[:, t, :], sb_w1[:, ts(t)], devt, start=False, stop=True)

    # h, d, hd, e in column form [128, tile, sample]
    h_c = work.tile([128, 2, BLOC], F32)
    for t in range(2):
        nc.scalar.activation(
            out=h_c[:, t, :], in_=p_uc[:, t, :], func=AF.Tanh,
            bias=sb_b1c[:, t : t + 1], scale=1.0,
        )
    d_c = work.tile([128, 2, BLOC], F32)
    nc.vector.tensor_mul(d_c, h_c, h_c)
    nc.vector.tensor_scalar(
        out=d_c, in0=d_c, scalar1=-1.0, scalar2=1.0, op0=OP.mult, op1=OP.add
    )
    hd_c = work.tile([128, 2, BLOC], F32)
    nc.vector.tensor_mul(hd_c, h_c, d_c)
    e_c = work.tile([128, 2, BLOC], F32)
    nc.vector.tensor_scalar_mul(e_c, hd_c, -2.0)

    if stage < 3:
        debug_out()
        return

    # ---------------- S1/S2 small matvecs (both samples batched) -------
    p_g = psmall.tile([128, 2, BLOC], F32, tag="small")
    p_w = psmall.tile([128, 2, BLOC], F32, tag="small")
    for t in range(2):
        nc.tensor.matmul(p_g[:, t, :], sb_w2t[:, ts(t)], velt, start=True, stop=True)
        nc.tensor.matmul(p_w[:, t, :], sb_w1[:, ts(t)], velt, start=True, stop=True)
    dg = work.tile([128, 2, BLOC], F32)
    nc.vector.tensor_mul(dg, p_g, d_c)            # d.*g
    eg = work.tile([128, 2, BLOC], F32)
    nc.vector.tensor_mul(eg, p_g, e_c)            # e.*g
    yv = work.tile([128, 2, BLOC], F32)
    nc.vector.tensor_mul(yv, eg, p_w)             # e.*g.*w
    p_cv = psmall.tile([128, 2, BLOC], F32, tag="small")
    p_ky = psmall.tile([128, 2, BLOC], F32, tag="small")
    for mt in range(2):
        for qt in range(2):
            nc.tensor.matmul(
                p_cv[:, mt, :], sb_K[:, qt, ts(mt)], dg[:, qt, :],
                start=(qt == 0), stop=(qt == 1),
            )
    for mt in range(2):
        for qt in range(2):
            nc.tensor.matmul(
                p_ky[:, mt, :], sb_K[:, qt, ts(mt)], yv[:, qt, :],
                start=(qt == 0), stop=(qt == 1),
            )
    z2 = work.tile([128, 2, BLOC], F32)
    nc.vector.tensor_mul(z2, eg, p_cv)            # e.*g.*cv
    i1 = work.tile([128, 2, BLOC], F32)
    nc.vector.tensor_mul(i1, e_c, p_cv)           # e.*cv
    nc.vector.tensor_mul(i1, i1, p_w)             # e.*cv.*w
    i2 = work.tile([128, 2, BLOC], F32)
    nc.vector.tensor_mul(i2, d_c, p_ky)           # d.*(K y)
    inner = work.tile([128, 2, BLOC], F32)
    nc.vector.tensor_add(inner, i1, i2)

    # S1 rows -> psum[:, 0:128], S2 rows (=0.5*S2) -> psum[:, 128:256]
    p_s12 = prow.tile([BLOC, 256], F32, tag="rows")
    for qt in range(2):
        nc.tensor.matmul(
            p_s12[:, 0:128], inner[:, qt, :], sb_w2[:, qt, :],
            start=(qt == 0), stop=(qt == 1),
        )
    for qt in range(2):
        nc.tensor.matmul(
            p_s12[:, 128:256], z2[:, qt, :], sb_w1t[:, ts(qt)],
            start=(qt == 0), stop=(qt == 1),
        )

    # ||v||^2 via ACT square-accumulate; restoration rows
    vsq = work.tile([BLOC, N], F32)
    svals = work.tile([BLOC, 2], F32)  # col0 = ||dG||^2, col1 = ||v||^2
    nc.scalar.activation(
        out=vsq, in_=sb_vel, func=AF.Square, accum_out=svals[:, 1:2]
    )
    rest = work.tile([BLOC, N], F32)
    nc.scalar.mul(out=rest, in_=sb_dev, mul=-0.1)

    if stage < 4:
        debug_out()
        return

    # ---------------- per-sample norm path ----------------
    # sub-bisect levels: 40=kd, 41=+T, 42=+ThK, 43=+U/Q, 44=+TT/TTR, 45=+Qe
    sub = stage - 40 if stage < 50 else 99

    acc_cols = work.tile([128, BLOC], F32)  # per-sample partial-sum columns
    for s in range(BLOC):
        # Kd = diag(d) K
        kd = loop.tile([128, 2, H], F32, tag="kd")
        for t in range(2):
            nc.scalar.activation(
                out=kd[:, t, :], in_=sb_K[:, t, :], func=AF.Copy,
                scale=d_c[:, t, s : s + 1],
            )
        if sub < 1:
            continue
        # T' = L @ Kd ; T = diag(d) T' ; Th = diag(h*d) T'
        T_sb = loop.tile([128, 2, H], F32, tag="T_sb")
        Th_sb = loop.tile([128, 2, H], F32, tag="Th_sb")
        for pt in range(2):
            p_tp = pbig.tile([128, 256], F32, tag="big")
            for qt in range(2):
                nc.tensor.matmul(
                    p_tp, _mm_dt(sb_L[:, qt, ts(pt)]), _mm_dt(kd[:, qt, :]),
                    start=(qt == 0), stop=(qt == 1),
                )
            nc.scalar.activation(
                out=T_sb[:, pt, :], in_=p_tp, func=AF.Copy,
                scale=d_c[:, pt, s : s + 1],
            )
            nc.scalar.activation(
                out=Th_sb[:, pt, :], in_=p_tp, func=AF.Copy,
                scale=hd_c[:, pt, s : s + 1],
            )
        if sub < 2:
            continue
        # ThK = Th .* K  (gpsimd, SBUF-only)
        ThK = loop.tile([128, 2, H], F32, tag="ThK")
        for t in range(2):
            nc.gpsimd.tensor_mul(ThK[:, t, :], Th_sb[:, t, :], sb_K[:, t, :])
        if sub < 3:
            continue
        # U = K @ T ; Q = KL .* U
        q_sb = loop.tile([128, 2, H], F32, tag="q_sb")
        for pt in range(2):
            p_u = pbig.tile([128, 256], F32, tag="big")
            for qt in range(2):
                nc.tensor.matmul(
                    p_u, _mm_dt(sb_K[:, qt, ts(pt)]), _mm_dt(T_sb[:, qt, :]),
                    start=(qt == 0), stop=(qt == 1),
                )
            nc.vector.tensor_mul(q_sb[:, pt, :], p_u, sb_KL[:, pt, :])
        if sub < 4:
            continue
        # term2: sum over tiles of ThK .* Th^T  (x4 applied in combine)
        ch = loop.tile([128, 3], F32, tag="ch")
        for pt in range(2):
            p_tt = pbig.tile([128, 256], F32, tag="big")
            for qt in range(2):
                nc.tensor.transpose(
                    out=p_tt[:, ts(qt)], in_=Th_sb[:, qt, ts(pt)], identity=ident
                )
            if sub < 5:
                continue
            scr = loop.tile([128, 256], F32, tag="scr")
            nc.vector.tensor_mul(scr, ThK[:, pt, :], p_tt)
            nc.vector.reduce_sum(ch[:, pt : pt + 1], scr, axis=AX.X)
        if sub < 6:
            continue
        # term1: e^T Q e  via PE matvec (Q symmetric) + weighted reduce
        p_qe = psmall.tile([128, BLOC], F32, tag="small")
        for mt in range(2):
            for qt in range(2):
                nc.tensor.matmul(
                    p_qe[:, mt : mt + 1], q_sb[:, qt, ts(mt)], e_c[:, qt, s : s + 1],
                    start=(qt == 0), stop=(qt == 1),
                )
        scr2 = loop.tile([128, 2], F32, tag="scr2")
        nc.vector.tensor_mul(scr2, p_qe, e_c[:, :, s])
        nc.vector.reduce_sum(ch[:, 2:3], scr2, axis=AX.X)
        # acc = 4*(ch0 + ch1) + ch2
        c01 = loop.tile([128, 1], F32, tag="c01")
        nc.vector.tensor_add(c01, ch[:, 0:1], ch[:, 1:2])
        nc.vector.scalar_tensor_tensor(
            out=acc_cols[:, s : s + 1], in0=c01, scalar=4.0, in1=ch[:, 2:3],
            op0=OP.mult, op1=OP.add,
        )

    if stage < 99:
        debug_out()
        return

    # ---------------- final scalars & output ----------------
    p_sc = psmall.tile([BLOC, 1], F32, tag="small")
    nc.tensor.matmul(p_sc, acc_cols, ones_col, start=True, stop=True)
    # svals col0 = ||dG||^2 = 2 * acc
    nc.scalar.mul(out=svals[:, 0:1], in_=p_sc, mul=2.0)

    # sqrt via bit trick + 2 Newton steps (avoids 2nd ACT table load)
    y = work.tile([BLOC, 2], F32)
    nc.vector.tensor_scalar(
        out=y.bitcast(I32), in0=svals.bitcast(I32),
        scalar1=1, scalar2=None, op0=OP.arith_shift_right,
    )
    nc.vector.tensor_scalar(
        out=y.bitcast(I32), in0=y.bitcast(I32),
        scalar1=SQRT_MAGIC, scalar2=None, op0=OP.add,
    )
    rcp = work.tile([BLOC, 2], F32)
    qn = work.tile([BLOC, 2], F32)
    for _ in range(2):
        nc.vector.reciprocal(rcp, y)
        nc.vector.tensor_mul(qn, svals, rcp)          # s / y
        nc.vector.tensor_add(y, y, qn)                # y + s/y
        nc.vector.tensor_scalar_mul(y, y, 0.5)        # 0.5*(y + s/y)
    # (||dG||+1e-6)*(||v||+1e-6), inverted
    nc.vector.tensor_scalar_add(y, y, 1e-6)
    den = work.tile([BLOC, 1], F32)
    nc.vector.tensor_mul(den, y[:, 0:1], y[:, 1:2])
    inv = work.tile([BLOC, 1], F32)
    nc.vector.reciprocal(inv, den)

    # a = (0.5*S2 - S1) * inv ; out_bot = a + (-0.1*dev)
    # p_s12[:,128:256] = W1 @ z2 which already equals 0.5*S2
    s2h = work.tile([BLOC, N], F32)
    nc.scalar.copy(out=s2h, in_=p_s12[:, 128:256])
    comb = work.tile([BLOC, N], F32)
    nc.vector.tensor_sub(comb, s2h, p_s12[:, 0:128])
    bot = work.tile([BLOC, N], F32)
    nc.vector.scalar_tensor_tensor(
        out=bot, in0=comb, scalar=inv, in1=rest, op0=OP.mult, op1=OP.add
    )
    nc.sync.dma_start(out=d_out[:, :], in_=bot)


_NC_CACHE = None


def _get_nc():
    global _NC_CACHE
    if _NC_CACHE is None:
        _NC_CACHE = build_nc()
    return _NC_CACHE


def make_in_maps(inputs):
    """Shard full inputs into per-core input maps."""
    state = np.ascontiguousarray(np.asarray(inputs["state_batch"], dtype=np.float32))
    x0 = np.ascontiguousarray(np.asarray(inputs["x0_batch"], dtype=np.float32))
    x1 = np.ascontiguousarray(np.asarray(inputs["x1_batch"], dtype=np.float32))
    W1 = np.ascontiguousarray(np.asarray(inputs["W1"], dtype=np.float32))
    W2 = np.ascontiguousarray(np.asarray(inputs["W2"], dtype=np.float32))
    b1 = np.ascontiguousarray(np.asarray(inputs["b1"], dtype=np.float32).reshape(1, H))
    t = np.ascontiguousarray(np.asarray(inputs["t"], dtype=np.float32).reshape(1, 1))
    dev, vel = state[:B], state[B:]
    in_maps = []
    for c in range(NCORES):
        sl = slice(c * BLOC, (c + 1) * BLOC)
        in_maps.append(
            {
                "x0": np.ascontiguousarray(x0[sl]),
                "x1": np.ascontiguousarray(x1[sl]),
                "dev": np.ascontiguousarray(dev[sl]),
                "vel": np.ascontiguousarray(vel[sl]),
                "W1": W1,
                "W2": W2,
                "b1": b1,
                "t": t,
            }
        )
    return in_maps, vel


def kernel(**inputs) -> np.ndarray:
    from concourse.bass_utils import run_bass_kernel_spmd

    nc = _get_nc()
    in_maps, vel = make_in_maps(inputs)
    res = run_bass_kernel_spmd(nc, in_maps, core_ids=list(range(NCORES)))
    bottom = np.concatenate([res.results[c]["out_bot"] for c in range(NCORES)], axis=0)
    return np.concatenate([vel, bottom], axis=0).astype(np.float32)


# revision 16
# speedup vs baseline: 1.0733x; 1.0733x over previous
"""Trainium2 Bass kernel for BatchedModelManifoldGeodesicFlow.

Closed-form math (per sample), derived from the reference's autodiff:
  f(x) = tanh(x@W1 + b1)@W2 + b2,  J = jacrev(f)(x) = W2^T diag(d) W1^T
  with h = tanh(x@W1+b1), d = 1-h^2, e = -2*h*d, K = W1^T W1, L = W2 W2^T.
  V := L diag(d) K,  W := K diag(d) L (= V^T),  U := K diag(d) V,
  Q := (K.*L).*U,    R := K.*V.*W          (Q, R symmetric)
    ||dG||^2 = 2*( e^T Q e + 4 * (h.*d)^T R (h.*d) )
  Christoffel contraction -> small matvecs:
    S1 = W2^T [ e.*cv.*w + d.*(K (e.*g.*w)) ],  0.5*S2 = W1 (e.*g.*cv)
    w = W1^T v, g = W2 v, cv = K (d.*g)
    a = (0.5*S2 - S1) / ((||dG||+1e-6) * (||v||+1e-6))
  out = concat([v, a - 0.1*dev], axis=0)

Sharding: pure data parallel, batch 16 -> 2 samples per core on 8 cores.
Big [256x256x256] matmuls run as float32r (PE fast fp32 mode); their
operands are pre-rounded to f32r by the producing copy instructions.
"""

import sys

if "/opt/trn_rl_repo" not in sys.path:
    sys.path.insert(0, "/opt/trn_rl_repo")

import numpy as np

import concourse.bacc as bacc
import concourse.tile as tile
from concourse import mybir
from concourse.masks import make_identity

N = 128
H = 256
B = 16
NCORES = 8
BLOC = B // NCORES  # 2 samples per core

F32 = mybir.dt.float32
F32R = mybir.dt.float32r
I32 = mybir.dt.int32
AF = mybir.ActivationFunctionType
OP = mybir.AluOpType
AX = mybir.AxisListType

SQRT_MAGIC = 0x1FBD1DF5  # bits trick: sqrt(x) ~ bitcast((bits(x)>>1) + MAGIC)


def ts(i, sz=128):
    return slice(i * sz, (i + 1) * sz)


def build_nc():
    nc = bacc.Bacc(trn_type="TRN2", enable_partition_id=False)

    d_vecs = nc.dram_tensor("vecs", [4 * BLOC, N], F32, kind="ExternalInput")
    d_w1 = nc.dram_tensor("W1", [N, H], F32, kind="ExternalInput")
    d_w2 = nc.dram_tensor("W2", [H, N], F32, kind="ExternalInput")
    d_b1 = nc.dram_tensor("b1", [1, H], F32, kind="ExternalInput")
    d_t = nc.dram_tensor("t", [1, 1], F32, kind="ExternalInput")
    d_out = nc.dram_tensor("out_bot", [BLOC, N], F32, kind="ExternalOutput")

    with tile.TileContext(nc) as tc:
        with (
            tc.tile_pool(name="consts", bufs=1) as consts,
            tc.tile_pool(name="work", bufs=1) as work,
            tc.tile_pool(name="loop", bufs=2) as loop,
            tc.tile_pool(name="pbig", bufs=4, space="PSUM") as pbig,
            tc.tile_pool(name="psmall", bufs=3, space="PSUM") as psmall,
            tc.tile_pool(name="prow", bufs=1, space="PSUM") as prow,
        ):
            _emit(nc, consts, work, loop, pbig, psmall, prow,
                  d_vecs, d_w1, d_w2, d_b1, d_t, d_out)

    nc.compile()  # Bacc: split multi-waits into event semaphores, alloc regs
    return nc


def _emit(nc, consts, work, loop, pbig, psmall, prow,
          d_vecs, d_w1, d_w2, d_b1, d_t, d_out):
    # ---------------- input DMA ----------------
    sb_vecs = consts.tile([4 * BLOC, N], F32)   # rows: x0, x1, dev, vel
    dev_rows = consts.tile([BLOC, N], F32)
    vel_rows = consts.tile([BLOC, N], F32)
    sb_w1 = consts.tile([128, H], F32)          # W1[k, m]
    sb_w2 = consts.tile([128, 2, 128], F32)     # W2 tiles: [m%128, m//128, j]
    sb_b1row = consts.tile([1, H], F32)
    sb_t = consts.tile([1, 1], F32)
    nc.sync.dma_start(out=sb_vecs, in_=d_vecs[:, :])
    nc.sync.dma_start(out=dev_rows, in_=d_vecs[2 * BLOC : 3 * BLOC, :])
    nc.sync.dma_start(out=vel_rows, in_=d_vecs[3 * BLOC : 4 * BLOC, :])
    nc.sync.dma_start(out=sb_w1, in_=d_w1[:, :])
    nc.sync.dma_start(out=sb_w2, in_=d_w2.rearrange("(t p) n -> p t n", p=128))
    nc.sync.dma_start(out=sb_b1row, in_=d_b1[:, :])
    nc.sync.dma_start(out=sb_t, in_=d_t[:, :])

    # ---------------- constants ----------------
    ident = consts.tile([128, 128], F32)
    make_identity(nc, ident)
    ones_row = consts.tile([1, 128], F32)
    nc.vector.memset(ones_row, 1.0)
    ones_col = consts.tile([128, 1], F32)
    nc.vector.memset(ones_col, 1.0)

    # rounded weights for f32r matmuls
    w1_r = consts.tile([128, H], F32R)
    nc.vector.tensor_copy(w1_r, sb_w1)

    # W2^T [j, m] and W1^T blocks [m, i]
    sb_w2t = consts.tile([128, H], F32)
    w2t_r = consts.tile([128, H], F32R)
    p_w2t = pbig.tile([128, 256], F32, tag="big")
    for t in range(2):
        nc.tensor.transpose(out=p_w2t[:, ts(t)], in_=sb_w2[:, t, :], identity=ident)
    nc.scalar.copy(out=sb_w2t, in_=p_w2t)
    nc.vector.tensor_copy(w2t_r, p_w2t)

    sb_w1t = consts.tile([128, H], F32)  # block t = (W1[:, t*128:+128])^T
    p_w1t = pbig.tile([128, 256], F32, tag="big")
    for t in range(2):
        nc.tensor.transpose(out=p_w1t[:, ts(t)], in_=sb_w1[:, ts(t)], identity=ident)
    nc.scalar.copy(out=sb_w1t, in_=p_w1t)

    # b1 in column form [m%128, m//128]
    sb_b1c = consts.tile([128, 2], F32)
    p_b1c = psmall.tile([128, 2], F32, tag="small")
    for t in range(2):
        nc.tensor.transpose(
            out=p_b1c[:, t : t + 1], in_=sb_b1row[0:1, ts(t)], identity=ident[0:1, 0:1]
        )
    nc.scalar.copy(out=sb_b1c, in_=p_b1c)

    # K = W1^T W1, L = W2 W2^T (f32r fast mode), KL = K.*L
    sb_K = consts.tile([128, 2, H], F32)
    sb_Lr = consts.tile([128, 2, H], F32R)
    sb_L = consts.tile([128, 2, H], F32)
    sb_KL = consts.tile([128, 2, H], F32)
    for t in range(2):
        p_k = pbig.tile([128, 256], F32, tag="big")
        nc.tensor.matmul(p_k, w1_r[:, ts(t)], w1_r, start=True, stop=True)
        nc.scalar.copy(out=sb_K[:, t, :], in_=p_k)
    for t in range(2):
        p_l = pbig.tile([128, 256], F32, tag="big")
        nc.tensor.matmul(p_l, w2t_r[:, ts(t)], w2t_r, start=True, stop=True)
        nc.scalar.copy(out=sb_L[:, t, :], in_=p_l)
        nc.vector.tensor_copy(sb_Lr[:, t, :], p_l)
    for t in range(2):
        nc.vector.tensor_mul(sb_KL[:, t, :], sb_K[:, t, :], sb_L[:, t, :])

    # ---------------- t scalar / window ----------------
    p_t128 = psmall.tile([128, 1], F32, tag="small")
    nc.tensor.matmul(p_t128, ones_row[0:1, :], sb_t[0:1, 0:1], start=True, stop=True)
    t128 = work.tile([128, 1], F32)
    nc.scalar.copy(out=t128, in_=p_t128)
    omt = work.tile([128, 1], F32)
    nc.vector.tensor_scalar(
        out=omt, in0=p_t128, scalar1=-1.0, scalar2=1.0, op0=OP.mult, op1=OP.add
    )
    wf4 = work.tile([128, 1], F32)
    nc.vector.tensor_mul(wf4, omt, t128)                       # t*(1-t)
    nc.vector.tensor_scalar_mul(wf4, wf4, 4.0)                 # 4t(1-t)

    # ---------------- columns of x0/x1/dev/vel; x in column space ------
    p_vt = psmall.tile([128, 4 * BLOC], F32, tag="small")
    nc.tensor.transpose(
        out=p_vt, in_=sb_vecs, identity=ident[0 : 4 * BLOC, 0 : 4 * BLOC]
    )
    vc = work.tile([128, 4 * BLOC], F32)   # cols: x0 | x1 | dev | vel
    nc.scalar.copy(out=vc, in_=p_vt)
    x0c, x1c = vc[:, 0:2], vc[:, 2:4]
    devc, velc = vc[:, 4:6], vc[:, 6:8]

    dxc = work.tile([128, BLOC], F32)
    nc.vector.tensor_sub(dxc, x1c, x0c)
    xc = work.tile([128, BLOC], F32)
    nc.vector.scalar_tensor_tensor(
        out=xc, in0=dxc, scalar=t128, in1=x0c, op0=OP.mult, op1=OP.add
    )
    nc.vector.scalar_tensor_tensor(
        out=xc, in0=devc, scalar=wf4, in1=xc, op0=OP.mult, op1=OP.add
    )

    # u columns per H-tile: [m%128, tile, sample]; h = tanh(u + b1)
    p_uc = psmall.tile([128, 2, BLOC], F32, tag="small")
    for t in range(2):
        nc.tensor.matmul(p_uc[:, t, :], sb_w1[:, ts(t)], xc, start=True, stop=True)
    h_c = work.tile([128, 2, BLOC], F32)
    for t in range(2):
        nc.scalar.activation(
            out=h_c[:, t, :], in_=p_uc[:, t, :], func=AF.Tanh,
            bias=sb_b1c[:, t : t + 1], scale=1.0,
        )
    d_c = work.tile([128, 2, BLOC], F32)
    nc.vector.tensor_mul(d_c, h_c, h_c)
    nc.vector.tensor_scalar(
        out=d_c, in0=d_c, scalar1=-1.0, scalar2=1.0, op0=OP.mult, op1=OP.add
    )
    hd_c = work.tile([128, 2, BLOC], F32)
    nc.vector.tensor_mul(hd_c, h_c, d_c)
    e_c = work.tile([128, 2, BLOC], F32)
    nc.vector.tensor_scalar_mul(e_c, hd_c, -2.0)

    # ---------------- S1/S2 small matvecs (both samples batched) -------
    p_gw = psmall.tile([128, 2, 2, BLOC], F32, tag="small")  # [t, {g,w}, s]
    for t in range(2):
        nc.tensor.matmul(p_gw[:, t, 0, :], sb_w2t[:, ts(t)], velc, start=True, stop=True)
        nc.tensor.matmul(p_gw[:, t, 1, :], sb_w1[:, ts(t)], velc, start=True, stop=True)
    p_g = p_gw[:, :, 0, :]
    p_w = p_gw[:, :, 1, :]
    dgy = work.tile([128, 2, 2, BLOC], F32)  # [qt, {dg,yv}, s]
    eg = work.tile([128, 2, BLOC], F32)
    nc.vector.tensor_mul(dgy[:, :, 0, :], p_g, d_c)          # d.*g
    nc.vector.tensor_mul(eg, p_g, e_c)                       # e.*g
    nc.vector.tensor_mul(dgy[:, :, 1, :], eg, p_w)           # e.*g.*w
    p_cvky = psmall.tile([128, 2, 2, BLOC], F32, tag="small")  # [mt, {cv,ky}, s]
    for mt in range(2):
        for qt in range(2):
            nc.tensor.matmul(
                p_cvky[:, mt, :, :], sb_K[:, qt, ts(mt)], dgy[:, qt, :, :],
                start=(qt == 0), stop=(qt == 1),
            )
    p_cv = p_cvky[:, :, 0, :]
    p_ky = p_cvky[:, :, 1, :]
    z2 = work.tile([128, 2, BLOC], F32)
    nc.vector.tensor_mul(z2, eg, p_cv)            # e.*g.*cv
    i1 = work.tile([128, 2, BLOC], F32)
    nc.vector.tensor_mul(i1, e_c, p_cv)           # e.*cv
    nc.vector.tensor_mul(i1, i1, p_w)             # e.*cv.*w
    i2 = work.tile([128, 2, BLOC], F32)
    nc.vector.tensor_mul(i2, d_c, p_ky)           # d.*(K y)
    inner = work.tile([128, 2, BLOC], F32)
    nc.vector.tensor_add(inner, i1, i2)

    # S1 rows -> psum[:, 0:128], 0.5*S2 rows -> psum[:, 128:256]
    p_s12 = prow.tile([BLOC, 256], F32, tag="rows")
    for qt in range(2):
        nc.tensor.matmul(
            p_s12[:, 0:128], inner[:, qt, :], sb_w2[:, qt, :],
            start=(qt == 0), stop=(qt == 1),
        )
    for qt in range(2):
        nc.tensor.matmul(
            p_s12[:, 128:256], z2[:, qt, :], sb_w1t[:, ts(qt)],
            start=(qt == 0), stop=(qt == 1),
        )

    # ||v||^2 via ACT square-accumulate; restoration rows
    vsq = work.tile([BLOC, N], F32)
    svals = work.tile([BLOC, 2], F32)  # col0 = ||dG||^2, col1 = ||v||^2
    nc.scalar.activation(
        out=vsq, in_=vel_rows, func=AF.Square, accum_out=svals[:, 1:2]
    )
    rest = work.tile([BLOC, N], F32)
    nc.scalar.mul(out=rest, in_=dev_rows, mul=-0.1)

    # ---------------- per-sample norm path ----------------
    acc_cols = work.tile([128, BLOC], F32)
    for s in range(BLOC):
        # Kd = diag(d) K, rounded for f32r matmuls
        kd_r = loop.tile([128, 2, H], F32R, tag="kd")
        for t in range(2):
            nc.scalar.activation(
                out=kd_r[:, t, :], in_=sb_K[:, t, :], func=AF.Copy,
                scale=d_c[:, t, s : s + 1],
            )
        # V = L diag(d) K  (psum) -> v_r rounded sbuf copy
        v_r = loop.tile([128, 2, H], F32R, tag="v_r")
        for pt in range(2):
            p_v = pbig.tile([128, 256], F32, tag="big")
            for qt in range(2):
                nc.tensor.matmul(
                    p_v, sb_Lr[:, qt, ts(pt)], kd_r[:, qt, :],
                    start=(qt == 0), stop=(qt == 1),
                )
            nc.scalar.copy(out=v_r[:, pt, :], in_=p_v)
        # vk = V .* K on gpsimd (SBUF-only)
        vk = loop.tile([128, 2, H], F32, tag="vk")
        for t in range(2):
            nc.gpsimd.tensor_mul(vk[:, t, :], v_r.bitcast(F32)[:, t, :], sb_K[:, t, :])
        # W = K diag(d) L (psum); R = vk .* W ; U = K diag(d) V ; Q = KL .* U
        q_sb = loop.tile([128, 2, H], F32, tag="q_sb")
        r_sb = loop.tile([128, 2, H], F32, tag="r_sb")
        for pt in range(2):
            p_w2m = pbig.tile([128, 256], F32, tag="big")
            for qt in range(2):
                nc.tensor.matmul(
                    p_w2m, kd_r[:, qt, ts(pt)], sb_Lr[:, qt, :],
                    start=(qt == 0), stop=(qt == 1),
                )
            nc.vector.tensor_mul(r_sb[:, pt, :], vk[:, pt, :], p_w2m)
        for pt in range(2):
            p_u = pbig.tile([128, 256], F32, tag="big")
            for qt in range(2):
                nc.tensor.matmul(
                    p_u, kd_r[:, qt, ts(pt)], v_r[:, qt, :],
                    start=(qt == 0), stop=(qt == 1),
                )
            nc.vector.tensor_mul(q_sb[:, pt, :], p_u, sb_KL[:, pt, :])
        # matvecs: Qe and R(hd) (both matrices symmetric)
        p_qr = psmall.tile([128, 2, 2], F32, tag="small")  # [{q,r}, mt]
        for mt in range(2):
            for qt in range(2):
                nc.tensor.matmul(
                    p_qr[:, 0, mt : mt + 1], q_sb[:, qt, ts(mt)],
                    e_c[:, qt, s : s + 1], start=(qt == 0), stop=(qt == 1),
                )
            for qt in range(2):
                nc.tensor.matmul(
                    p_qr[:, 1, mt : mt + 1], r_sb[:, qt, ts(mt)],
                    hd_c[:, qt, s : s + 1], start=(qt == 0), stop=(qt == 1),
                )
        ch = loop.tile([128, 2], F32, tag="ch")
        scrq = loop.tile([128, 2], F32, tag="scrq")
        nc.vector.tensor_mul(scrq, p_qr[:, 0, :], e_c[:, :, s])
        nc.vector.reduce_sum(ch[:, 0:1], scrq, axis=AX.X)
        scrr = loop.tile([128, 2], F32, tag="scrr")
        nc.vector.tensor_mul(scrr, p_qr[:, 1, :], hd_c[:, :, s])
        nc.vector.reduce_sum(ch[:, 1:2], scrr, axis=AX.X)
        # acc = term1 + 4*term2
        nc.vector.scalar_tensor_tensor(
            out=acc_cols[:, s : s + 1], in0=ch[:, 1:2], scalar=4.0, in1=ch[:, 0:1],
            op0=OP.mult, op1=OP.add,
        )

    # ---------------- final scalars & output ----------------
    p_sc = psmall.tile([BLOC, 1], F32, tag="small")
    nc.tensor.matmul(p_sc, acc_cols, ones_col, start=True, stop=True)
    nc.scalar.mul(out=svals[:, 0:1], in_=p_sc, mul=2.0)  # ||dG||^2

    # sqrt via bit trick + 2 Newton steps (avoids 2nd ACT table load)
    y = work.tile([BLOC, 2], F32)
    nc.vector.tensor_scalar(
        out=y.bitcast(I32), in0=svals.bitcast(I32),
        scalar1=1, scalar2=None, op0=OP.arith_shift_right,
    )
    nc.vector.tensor_scalar(
        out=y.bitcast(I32), in0=y.bitcast(I32),
        scalar1=SQRT_MAGIC, scalar2=None, op0=OP.add,
    )
    rcp = work.tile([BLOC, 2], F32)
    qn = work.tile([BLOC, 2], F32)
    for _ in range(2):
        nc.vector.reciprocal(rcp, y)
        nc.vector.tensor_mul(qn, svals, rcp)          # s / y
        nc.vector.tensor_add(y, y, qn)                # y + s/y
        nc.vector.tensor_scalar_mul(y, y, 0.5)        # 0.5*(y + s/y)
    nc.vector.tensor_scalar_add(y, y, 1e-6)
    den = work.tile([BLOC, 1], F32)
    nc.vector.tensor_mul(den, y[:, 0:1], y[:, 1:2])
    inv = work.tile([BLOC, 1], F32)
    nc.vector.reciprocal(inv, den)

    # a = (0.5*S2 - S1) * inv ; out_bot = a + (-0.1*dev)
    s2h = work.tile([BLOC, N], F32)
    nc.scalar.copy(out=s2h, in_=p_s12[:, 128:256])
    comb = work.tile([BLOC, N], F32)
    nc.vector.tensor_sub(comb, s2h, p_s12[:, 0:128])
    bot = work.tile([BLOC, N], F32)
    nc.vector.scalar_tensor_tensor(
        out=bot, in0=comb, scalar=inv, in1=rest, op0=OP.mult, op1=OP.add
    )
    nc.sync.dma_start(out=d_out[:, :], in_=bot)


_NC_CACHE = None


def _get_nc():
    global _NC_CACHE
    if _NC_CACHE is None:
        _NC_CACHE = build_nc()
    return _NC_CACHE


def make_in_maps(inputs):
    """Shard full inputs into per-core input maps."""
    state = np.ascontiguousarray(np.asarray(inputs["state_batch"], dtype=np.float32))
    x0 = np.asarray(inputs["x0_batch"], dtype=np.float32)
    x1 = np.asarray(inputs["x1_batch"], dtype=np.float32)
    W1 = np.ascontiguousarray(np.asarray(inputs["W1"], dtype=np.float32))
    W2 = np.ascontiguousarray(np.asarray(inputs["W2"], dtype=np.float32))
    b1 = np.ascontiguousarray(np.asarray(inputs["b1"], dtype=np.float32).reshape(1, H))
    t = np.ascontiguousarray(np.asarray(inputs["t"], dtype=np.float32).reshape(1, 1))
    dev, vel = state[:B], state[B:]
    in_maps = []
    for c in range(NCORES):
        sl = slice(c * BLOC, (c + 1) * BLOC)
        vecs = np.concatenate([x0[sl], x1[sl], dev[sl], vel[sl]], axis=0)
        in_maps.append(
            {
                "vecs": np.ascontiguousarray(vecs),
                "W1": W1,
                "W2": W2,
                "b1": b1,
                "t": t,
            }
        )
    return in_maps, vel


def kernel(**inputs) -> np.ndarray:
    from concourse.bass_utils import run_bass_kernel_spmd

    nc = _get_nc()
    in_maps, vel = make_in_maps(inputs)
    res = run_bass_kernel_spmd(nc, in_maps, core_ids=list(range(NCORES)))
    bottom = np.concatenate([res.results[c]["out_bot"] for c in range(NCORES)], axis=0)
    return np.concatenate([vel, bottom], axis=0).astype(np.float32)


# revision 18
# speedup vs baseline: 1.2110x; 1.1282x over previous
"""Trainium2 Bass kernel for BatchedModelManifoldGeodesicFlow.

Closed-form math (per sample), derived from the reference's autodiff:
  f(x) = tanh(x@W1 + b1)@W2 + b2,  J = jacrev(f)(x) = W2^T diag(d) W1^T
  with h = tanh(x@W1+b1), d = 1-h^2, e = -2*h*d, K = W1^T W1, L = W2 W2^T.
  V := L diag(d) K,  W := K diag(d) L (= V^T),  U := K diag(d) V,
  Q := (K.*L).*U,    R := K.*V.*W          (Q, R symmetric)
    ||dG||^2 = 2*( e^T Q e + 4 * (h.*d)^T R (h.*d) )
  Christoffel contraction -> small matvecs:
    S1 = W2^T [ e.*cv.*w + d.*(K (e.*g.*w)) ],  0.5*S2 = W1 (e.*g.*cv)
    w = W1^T v, g = W2 v, cv = K (d.*g)
    a = (0.5*S2 - S1) / ((||dG||+1e-6) * (||v||+1e-6))
  out = concat([v, a - 0.1*dev], axis=0)

Sharding: pure data parallel, batch 16 -> 2 samples per core on 8 cores.
Big [256x256x256] matmuls run as float32r (PE fast fp32 mode); their
operands are pre-rounded to f32r by the producing copy instructions.
"""

import sys

if "/opt/trn_rl_repo" not in sys.path:
    sys.path.insert(0, "/opt/trn_rl_repo")

import numpy as np

import concourse.bacc as bacc
import concourse.tile as tile
from concourse import mybir
from concourse.masks import make_identity

N = 128
H = 256
B = 16
NCORES = 8
BLOC = B // NCORES  # 2 samples per core

F32 = mybir.dt.float32
F32R = mybir.dt.float32r
I32 = mybir.dt.int32
AF = mybir.ActivationFunctionType
OP = mybir.AluOpType
AX = mybir.AxisListType

SQRT_MAGIC = 0x1FBD1DF5  # bits trick: sqrt(x) ~ bitcast((bits(x)>>1) + MAGIC)


def ts(i, sz=128):
    return slice(i * sz, (i + 1) * sz)


def build_nc():
    nc = bacc.Bacc(trn_type="TRN2", enable_partition_id=False)

    d_vecs = nc.dram_tensor("vecs", [4 * BLOC, N], F32, kind="ExternalInput")
    d_w1 = nc.dram_tensor("W1", [N, H], F32, kind="ExternalInput")
    d_w2 = nc.dram_tensor("W2", [H, N], F32, kind="ExternalInput")
    d_b1 = nc.dram_tensor("b1", [1, H], F32, kind="ExternalInput")
    d_t = nc.dram_tensor("t", [1, 1], F32, kind="ExternalInput")
    d_out = nc.dram_tensor("out_bot", [BLOC, N], F32, kind="ExternalOutput")

    with tile.TileContext(nc) as tc:
        with (
            tc.tile_pool(name="consts", bufs=1) as consts,
            tc.tile_pool(name="work", bufs=1) as work,
            tc.tile_pool(name="loop", bufs=2) as loop,
            tc.tile_pool(name="pbig", bufs=4, space="PSUM") as pbig,
            tc.tile_pool(name="psmall", bufs=3, space="PSUM") as psmall,
            tc.tile_pool(name="prow", bufs=1, space="PSUM") as prow,
        ):
            _emit(nc, consts, work, loop, pbig, psmall, prow,
                  d_vecs, d_w1, d_w2, d_b1, d_t, d_out)

    nc.compile()  # Bacc: split multi-waits into event semaphores, alloc regs
    return nc


def _emit(nc, consts, work, loop, pbig, psmall, prow,
          d_vecs, d_w1, d_w2, d_b1, d_t, d_out):
    # ---------------- input DMA ----------------
    sb_vecs = consts.tile([4 * BLOC, N], F32)   # rows: x0, x1, dev, vel
    dev_rows = consts.tile([BLOC, N], F32)
    vel_rows = consts.tile([BLOC, N], F32)
    sb_w1 = consts.tile([128, H], F32)          # W1[k, m]
    sb_w2 = consts.tile([128, 2, 128], F32)     # W2 tiles: [m%128, m//128, j]
    sb_b1row = consts.tile([1, H], F32)
    sb_t = consts.tile([1, 1], F32)
    nc.sync.dma_start(out=sb_vecs, in_=d_vecs[:, :])
    nc.sync.dma_start(out=dev_rows, in_=d_vecs[2 * BLOC : 3 * BLOC, :])
    nc.sync.dma_start(out=vel_rows, in_=d_vecs[3 * BLOC : 4 * BLOC, :])
    nc.sync.dma_start(out=sb_w1, in_=d_w1[:, :])
    nc.sync.dma_start(out=sb_w2, in_=d_w2.rearrange("(t p) n -> p t n", p=128))
    nc.sync.dma_start(out=sb_b1row, in_=d_b1[:, :])
    nc.sync.dma_start(out=sb_t, in_=d_t[:, :])

    # ---------------- constants ----------------
    ident = consts.tile([128, 128], F32)
    make_identity(nc, ident)
    ones_row = consts.tile([1, 128], F32)
    nc.vector.memset(ones_row, 1.0)
    ones_col = consts.tile([128, 1], F32)
    nc.vector.memset(ones_col, 1.0)
    ones2_r = consts.tile([128, 2], F32R)
    nc.vector.memset(ones2_r.bitcast(F32), 1.0)
    nc.vector.tensor_copy(ones2_r, ones2_r.bitcast(F32))

    # rounded weights for f32r matmuls
    w1_r = consts.tile([128, H], F32R)
    nc.vector.tensor_copy(w1_r, sb_w1)

    # W2^T [j, m] and W1^T blocks [m, i]
    sb_w2t = consts.tile([128, H], F32)
    w2t_r = consts.tile([128, H], F32R)
    p_w2t = pbig.tile([128, 256], F32, tag="big")
    for t in range(2):
        nc.tensor.transpose(out=p_w2t[:, ts(t)], in_=sb_w2[:, t, :], identity=ident)
    nc.scalar.copy(out=sb_w2t, in_=p_w2t)
    nc.vector.tensor_copy(w2t_r, p_w2t)

    w2_r = consts.tile([128, 2, 128], F32R)
    nc.vector.tensor_copy(w2_r, sb_w2)

    sb_w1t = consts.tile([128, H], F32R)  # block t = (W1[:, t*128:+128])^T
    p_w1t = pbig.tile([128, 256], F32, tag="big")
    for t in range(2):
        nc.tensor.transpose(out=p_w1t[:, ts(t)], in_=sb_w1[:, ts(t)], identity=ident)
    nc.scalar.copy(out=sb_w1t, in_=p_w1t)

    # b1 in column form [m%128, m//128]
    sb_b1c = consts.tile([128, 2], F32)
    p_b1c = psmall.tile([128, 2], F32, tag="small")
    for t in range(2):
        nc.tensor.transpose(
            out=p_b1c[:, t : t + 1], in_=sb_b1row[0:1, ts(t)], identity=ident[0:1, 0:1]
        )
    nc.scalar.copy(out=sb_b1c, in_=p_b1c)

    # K = W1^T W1, L = W2 W2^T (f32r fast mode), KL = K.*L
    sb_K = consts.tile([128, 2, H], F32R)
    sb_Lr = consts.tile([128, 2, H], F32R)
    sb_L = consts.tile([128, 2, H], F32)
    sb_KL = consts.tile([128, 2, H], F32)
    for t in range(2):
        p_k = pbig.tile([128, 256], F32, tag="big")
        nc.tensor.matmul(p_k, w1_r[:, ts(t)], w1_r, start=True, stop=True)
        nc.scalar.copy(out=sb_K[:, t, :], in_=p_k)
    for t in range(2):
        p_l = pbig.tile([128, 256], F32, tag="big")
        nc.tensor.matmul(p_l, w2t_r[:, ts(t)], w2t_r, start=True, stop=True)
        nc.scalar.copy(out=sb_L[:, t, :], in_=p_l)
        nc.vector.tensor_copy(sb_Lr[:, t, :], p_l)
    for t in range(2):
        nc.vector.tensor_mul(sb_KL[:, t, :], sb_K[:, t, :], sb_L[:, t, :])

    # ---------------- t scalar / window ----------------
    p_t128 = psmall.tile([128, 1], F32, tag="small")
    nc.tensor.matmul(p_t128, ones_row[0:1, :], sb_t[0:1, 0:1], start=True, stop=True)
    t128 = work.tile([128, 1], F32)
    nc.scalar.copy(out=t128, in_=p_t128)
    omt = work.tile([128, 1], F32)
    nc.vector.tensor_scalar(
        out=omt, in0=p_t128, scalar1=-1.0, scalar2=1.0, op0=OP.mult, op1=OP.add
    )
    wf4 = work.tile([128, 1], F32)
    nc.vector.tensor_mul(wf4, omt, t128)                       # t*(1-t)
    nc.vector.tensor_scalar_mul(wf4, wf4, 4.0)                 # 4t(1-t)

    # ---------------- columns of x0/x1/dev/vel; x in column space ------
    p_vt = psmall.tile([128, 4 * BLOC], F32, tag="small")
    nc.tensor.transpose(
        out=p_vt, in_=sb_vecs, identity=ident[0 : 4 * BLOC, 0 : 4 * BLOC]
    )
    vc = work.tile([128, 4 * BLOC], F32R)  # cols: x0 | x1 | dev | vel
    nc.scalar.copy(out=vc, in_=p_vt)
    x0c, x1c = vc[:, 0:2], vc[:, 2:4]
    devc, velc = vc[:, 4:6], vc[:, 6:8]

    dxc = work.tile([128, BLOC], F32R)
    nc.vector.tensor_sub(dxc, x1c, x0c)
    xc = work.tile([128, BLOC], F32R)
    nc.vector.scalar_tensor_tensor(
        out=xc, in0=dxc, scalar=t128, in1=x0c, op0=OP.mult, op1=OP.add
    )
    nc.vector.scalar_tensor_tensor(
        out=xc, in0=devc, scalar=wf4, in1=xc, op0=OP.mult, op1=OP.add
    )

    # u columns per H-tile: [m%128, tile, sample]; h = tanh(u + b1)
    p_uc = psmall.tile([128, 2, BLOC], F32, tag="small")
    for t in range(2):
        nc.tensor.matmul(p_uc[:, t, :], w1_r[:, ts(t)], xc, start=True, stop=True)
    h_c = work.tile([128, 2, BLOC], F32)
    for t in range(2):
        nc.scalar.activation(
            out=h_c[:, t, :], in_=p_uc[:, t, :], func=AF.Tanh,
            bias=sb_b1c[:, t : t + 1], scale=1.0,
        )
    d_c = work.tile([128, 2, BLOC], F32)
    nc.vector.tensor_mul(d_c, h_c, h_c)
    nc.vector.tensor_scalar(
        out=d_c, in0=d_c, scalar1=-1.0, scalar2=1.0, op0=OP.mult, op1=OP.add
    )
    ehd_c = work.tile([128, 2, BLOC, 2], F32R)  # [...,0]=e, [...,1]=h*d
    nc.vector.tensor_mul(ehd_c[:, :, :, 1], h_c, d_c)
    nc.vector.tensor_scalar_mul(ehd_c[:, :, :, 0], ehd_c[:, :, :, 1], -2.0)
    e_c = ehd_c[:, :, :, 0]
    hd_c = ehd_c[:, :, :, 1]

    # ---------------- S1/S2 small matvecs (both samples batched) -------
    p_gw = psmall.tile([128, 2, 2, BLOC], F32, tag="small")  # [t, {g,w}, s]
    for t in range(2):
        nc.tensor.matmul(p_gw[:, t, 0, :], w2t_r[:, ts(t)], velc, start=True, stop=True)
        nc.tensor.matmul(p_gw[:, t, 1, :], w1_r[:, ts(t)], velc, start=True, stop=True)
    p_g = p_gw[:, :, 0, :]
    p_w = p_gw[:, :, 1, :]
    dgy = work.tile([128, 2, 2, BLOC], F32R)  # [qt, {dg,yv}, s]
    eg = work.tile([128, 2, BLOC], F32)
    nc.vector.tensor_mul(dgy[:, :, 0, :], p_g, d_c)          # d.*g
    nc.vector.tensor_mul(eg, p_g, e_c)                       # e.*g
    nc.vector.tensor_mul(dgy[:, :, 1, :], eg, p_w)           # e.*g.*w
    p_cvky = psmall.tile([128, 2, 2, BLOC], F32, tag="small")  # [mt, {cv,ky}, s]
    for mt in range(2):
        for qt in range(2):
            nc.tensor.matmul(
                p_cvky[:, mt, :, :], sb_K[:, qt, ts(mt)], dgy[:, qt, :, :],
                start=(qt == 0), stop=(qt == 1),
            )
    p_cv = p_cvky[:, :, 0, :]
    p_ky = p_cvky[:, :, 1, :]
    z2 = work.tile([128, 2, BLOC], F32R)
    nc.vector.tensor_mul(z2, eg, p_cv)            # e.*g.*cv
    i1 = work.tile([128, 2, BLOC], F32R)
    nc.vector.tensor_mul(i1, e_c, p_cv)           # e.*cv
    nc.vector.tensor_mul(i1, i1, p_w)             # e.*cv.*w
    i2 = work.tile([128, 2, BLOC], F32R)
    nc.vector.tensor_mul(i2, d_c, p_ky)           # d.*(K y)
    inner = work.tile([128, 2, BLOC], F32R)
    nc.vector.tensor_add(inner, i1, i2)

    # S1 rows -> psum[:, 0:128], 0.5*S2 rows -> psum[:, 128:256]
    p_s12 = prow.tile([BLOC, 256], F32, tag="rows")
    for qt in range(2):
        nc.tensor.matmul(
            p_s12[:, 0:128], inner[:, qt, :], w2_r[:, qt, :],
            start=(qt == 0), stop=(qt == 1),
        )
    for qt in range(2):
        nc.tensor.matmul(
            p_s12[:, 128:256], z2[:, qt, :], sb_w1t[:, ts(qt)],
            start=(qt == 0), stop=(qt == 1),
        )

    # ||v||^2 via ACT square-accumulate; restoration rows
    vsq = work.tile([BLOC, N], F32)
    svals = work.tile([BLOC, 2], F32)  # col0 = ||dG||^2, col1 = ||v||^2
    nc.scalar.activation(
        out=vsq, in_=vel_rows, func=AF.Square, accum_out=svals[:, 1:2]
    )
    rest = work.tile([BLOC, N], F32)
    nc.scalar.mul(out=rest, in_=dev_rows, mul=-0.1)

    # ---------------- per-sample norm path ----------------
    acc_cols = work.tile([128, BLOC], F32R)
    for s in range(BLOC):
        # Kd = diag(d) K, rounded for f32r matmuls
        kd_r = loop.tile([128, 2, H], F32R, tag="kd")
        for t in range(2):
            nc.scalar.activation(
                out=kd_r[:, t, :], in_=sb_K[:, t, :], func=AF.Copy,
                scale=d_c[:, t, s : s + 1],
            )
        # V = L diag(d) K  (psum) -> v_r rounded sbuf copy
        v_r = loop.tile([128, 2, H], F32R, tag="v_r")
        for pt in range(2):
            p_v = pbig.tile([128, 256], F32, tag="big")
            for qt in range(2):
                nc.tensor.matmul(
                    p_v, sb_Lr[:, qt, ts(pt)], kd_r[:, qt, :],
                    start=(qt == 0), stop=(qt == 1),
                )
            nc.scalar.copy(out=v_r[:, pt, :], in_=p_v)
        # vk = V .* K on gpsimd (SBUF-only)
        vk = loop.tile([128, 2, H], F32, tag="vk")
        for t in range(2):
            nc.gpsimd.tensor_mul(vk[:, t, :], v_r.bitcast(F32)[:, t, :], sb_K[:, t, :])
        # W = K diag(d) L (psum); R = vk .* W ; U = K diag(d) V ; Q = KL .* U
        q_sb = loop.tile([128, 2, H], F32R, tag="q_sb")
        r_sb = loop.tile([128, 2, H], F32R, tag="r_sb")
        for pt in range(2):
            p_w2m = pbig.tile([128, 256], F32, tag="big")
            for qt in range(2):
                nc.tensor.matmul(
                    p_w2m, kd_r[:, qt, ts(pt)], sb_Lr[:, qt, :],
                    start=(qt == 0), stop=(qt == 1),
                )
            nc.vector.tensor_mul(r_sb[:, pt, :], vk[:, pt, :], p_w2m)
        for pt in range(2):
            p_u = pbig.tile([128, 256], F32, tag="big")
            for qt in range(2):
                nc.tensor.matmul(
                    p_u, kd_r[:, qt, ts(pt)], v_r[:, qt, :],
                    start=(qt == 0), stop=(qt == 1),
                )
            nc.vector.tensor_mul(q_sb[:, pt, :], p_u, sb_KL[:, pt, :])
        # matvecs: Qe and R(hd) (both matrices symmetric)
        p_qr = psmall.tile([128, 2, 2, 2], F32, tag="small")  # [{q,r}, mt, {e,hd}]
        for mt in range(2):
            for qt in range(2):
                nc.tensor.matmul(
                    p_qr[:, 0, mt, :], q_sb[:, qt, ts(mt)],
                    ehd_c[:, qt, s, :], start=(qt == 0), stop=(qt == 1),
                )
            for qt in range(2):
                nc.tensor.matmul(
                    p_qr[:, 1, mt, :], r_sb[:, qt, ts(mt)],
                    ehd_c[:, qt, s, :], start=(qt == 0), stop=(qt == 1),
                )
        ch = loop.tile([128, 2], F32, tag="ch")
        scrq = loop.tile([128, 2], F32, tag="scrq")
        nc.vector.tensor_mul(scrq, p_qr[:, 0, :, 0], e_c[:, :, s])
        nc.vector.reduce_sum(ch[:, 0:1], scrq, axis=AX.X)
        scrr = loop.tile([128, 2], F32, tag="scrr")
        nc.vector.tensor_mul(scrr, p_qr[:, 1, :, 1], hd_c[:, :, s])
        nc.vector.reduce_sum(ch[:, 1:2], scrr, axis=AX.X)
        # acc = term1 + 4*term2
        nc.vector.scalar_tensor_tensor(
            out=acc_cols[:, s : s + 1], in0=ch[:, 1:2], scalar=4.0, in1=ch[:, 0:1],
            op0=OP.mult, op1=OP.add,
        )

    # ---------------- final scalars & output ----------------
    p_sc = psmall.tile([BLOC, 2], F32, tag="small")
    nc.tensor.matmul(p_sc, acc_cols, ones2_r, start=True, stop=True)
    nc.scalar.mul(out=svals[:, 0:1], in_=p_sc[:, 0:1], mul=2.0)  # ||dG||^2

    # sqrt via bit trick + 2 Newton steps (avoids 2nd ACT table load)
    y = work.tile([BLOC, 2], F32)
    nc.vector.tensor_scalar(
        out=y.bitcast(I32), in0=svals.bitcast(I32),
        scalar1=1, scalar2=None, op0=OP.arith_shift_right,
    )
    nc.vector.tensor_scalar(
        out=y.bitcast(I32), in0=y.bitcast(I32),
        scalar1=SQRT_MAGIC, scalar2=None, op0=OP.add,
    )
    rcp = work.tile([BLOC, 2], F32)
    qn = work.tile([BLOC, 2], F32)
    for _ in range(2):
        nc.vector.reciprocal(rcp, y)
        nc.vector.tensor_mul(qn, svals, rcp)          # s / y
        nc.vector.tensor_add(y, y, qn)                # y + s/y
        nc.vector.tensor_scalar_mul(y, y, 0.5)        # 0.5*(y + s/y)
    nc.vector.tensor_scalar_add(y, y, 1e-6)
    den = work.tile([BLOC, 1], F32)
    nc.vector.tensor_mul(den, y[:, 0:1], y[:, 1:2])
    inv = work.tile([BLOC, 1], F32)
    nc.vector.reciprocal(inv, den)

    # a = (0.5*S2 - S1) * inv ; out_bot = a + (-0.1*dev)
    s2h = work.tile([BLOC, N], F32)
    nc.scalar.copy(out=s2h, in_=p_s12[:, 128:256])
    comb = work.tile([BLOC, N], F32)
    nc.vector.tensor_sub(comb, s2h, p_s12[:, 0:128])
    bot = work.tile([BLOC, N], F32)
    nc.vector.scalar_tensor_tensor(
        out=bot, in0=comb, scalar=inv, in1=rest, op0=OP.mult, op1=OP.add
    )
    nc.sync.dma_start(out=d_out[:, :], in_=bot)


_NC_CACHE = None


def _get_nc():
    global _NC_CACHE
    if _NC_CACHE is None:
        _NC_CACHE = build_nc()
    return _NC_CACHE


def make_in_maps(inputs):
    """Shard full inputs into per-core input maps."""
    state = np.ascontiguousarray(np.asarray(inputs["state_batch"], dtype=np.float32))
    x0 = np.asarray(inputs["x0_batch"], dtype=np.float32)
    x1 = np.asarray(inputs["x1_batch"], dtype=np.float32)
    W1 = np.ascontiguousarray(np.asarray(inputs["W1"], dtype=np.float32))
    W2 = np.ascontiguousarray(np.asarray(inputs["W2"], dtype=np.float32))
    b1 = np.ascontiguousarray(np.asarray(inputs["b1"], dtype=np.float32).reshape(1, H))
    t = np.ascontiguousarray(np.asarray(inputs["t"], dtype=np.float32).reshape(1, 1))
    dev, vel = state[:B], state[B:]
    in_maps = []
    for c in range(NCORES):
        sl = slice(c * BLOC, (c + 1) * BLOC)
        vecs = np.concatenate([x0[sl], x1[sl], dev[sl], vel[sl]], axis=0)
        in_maps.append(
            {
                "vecs": np.ascontiguousarray(vecs),
                "W1": W1,
                "W2": W2,
                "b1": b1,
                "t": t,
            }
        )
    return in_maps, vel


def kernel(**inputs) -> np.ndarray:
    from concourse.bass_utils import run_bass_kernel_spmd

    nc = _get_nc()
    in_maps, vel = make_in_maps(inputs)
    res = run_bass_kernel_spmd(nc, in_maps, core_ids=list(range(NCORES)))
    bottom = np.concatenate([res.results[c]["out_bot"] for c in range(NCORES)], axis=0)
    return np.concatenate([vel, bottom], axis=0).astype(np.float32)


# revision 21
# speedup vs baseline: 1.2247x; 1.0114x over previous
"""Trainium2 Bass kernel for BatchedModelManifoldGeodesicFlow.

Closed-form math (per sample), derived from the reference's autodiff:
  f(x) = tanh(x@W1 + b1)@W2 + b2,  J = jacrev(f)(x) = W2^T diag(d) W1^T
  with h = tanh(x@W1+b1), d = 1-h^2, e = -2*h*d, K = W1^T W1, L = W2 W2^T.
  V := L diag(d) K,  W := K diag(d) L (= V^T),  U := K diag(d) V,
  Q := (K.*L).*U,    R := K.*V.*W          (Q, R symmetric)
    ||dG||^2 = 2*( e^T Q e + 4 * (h.*d)^T R (h.*d) )
  Christoffel contraction -> small matvecs:
    S1 = W2^T [ e.*cv.*w + d.*(K (e.*g.*w)) ],  0.5*S2 = W1 (e.*g.*cv)
    w = W1^T v, g = W2 v, cv = K (d.*g)
    a = (0.5*S2 - S1) / ((||dG||+1e-6) * (||v||+1e-6))
  out = concat([v, a - 0.1*dev], axis=0)

Sharding: pure data parallel, batch 16 -> 2 samples per core on 8 cores.
Big [256x256x256] matmuls run as float32r (PE fast fp32 mode); their
operands are pre-rounded to f32r by the producing copy instructions.
"""

import sys

if "/opt/trn_rl_repo" not in sys.path:
    sys.path.insert(0, "/opt/trn_rl_repo")

import numpy as np

import concourse.bacc as bacc
import concourse.tile as tile
from concourse import mybir
from concourse.masks import make_identity

N = 128
H = 256
B = 16
NCORES = 8
BLOC = B // NCORES  # 2 samples per core

F32 = mybir.dt.float32
F32R = mybir.dt.float32r
I32 = mybir.dt.int32
AF = mybir.ActivationFunctionType
OP = mybir.AluOpType
AX = mybir.AxisListType

SQRT_MAGIC = 0x1FBD1DF5  # bits trick: sqrt(x) ~ bitcast((bits(x)>>1) + MAGIC)


def ts(i, sz=128):
    return slice(i * sz, (i + 1) * sz)


def build_nc():
    nc = bacc.Bacc(trn_type="TRN2", enable_partition_id=False)

    d_vecs = nc.dram_tensor("vecs", [4 * BLOC, N], F32, kind="ExternalInput")
    d_w1 = nc.dram_tensor("W1", [N, H], F32, kind="ExternalInput")
    d_w2 = nc.dram_tensor("W2", [H, N], F32, kind="ExternalInput")
    d_b1 = nc.dram_tensor("b1", [1, H], F32, kind="ExternalInput")
    d_t = nc.dram_tensor("t", [1, 1], F32, kind="ExternalInput")
    d_out = nc.dram_tensor("out_bot", [BLOC, N], F32, kind="ExternalOutput")

    with tile.TileContext(nc) as tc:
        with (
            tc.tile_pool(name="consts", bufs=1) as consts,
            tc.tile_pool(name="work", bufs=1) as work,
            tc.tile_pool(name="loop", bufs=2) as loop,
            tc.tile_pool(name="pbig", bufs=4, space="PSUM") as pbig,
            tc.tile_pool(name="psmall", bufs=3, space="PSUM") as psmall,
            tc.tile_pool(name="prow", bufs=1, space="PSUM") as prow,
        ):
            _emit(nc, consts, work, loop, pbig, psmall, prow,
                  d_vecs, d_w1, d_w2, d_b1, d_t, d_out)

    nc.compile()  # Bacc: split multi-waits into event semaphores, alloc regs
    return nc


def _emit(nc, consts, work, loop, pbig, psmall, prow,
          d_vecs, d_w1, d_w2, d_b1, d_t, d_out):
    # ---------------- input DMA ----------------
    sb_vecs = consts.tile([4 * BLOC, N], F32)   # rows: x0, x1, dev, vel
    dev_rows = consts.tile([BLOC, N], F32)
    vel_rows = consts.tile([BLOC, N], F32)
    sb_w1 = consts.tile([128, H], F32)          # W1[k, m]
    sb_w2 = consts.tile([128, 2, 128], F32)     # W2 tiles: [m%128, m//128, j]
    sb_b1row = consts.tile([1, H], F32)
    sb_t = consts.tile([1, 1], F32)
    nc.sync.dma_start(out=sb_vecs, in_=d_vecs[:, :])
    nc.sync.dma_start(out=dev_rows, in_=d_vecs[2 * BLOC : 3 * BLOC, :])
    nc.sync.dma_start(out=vel_rows, in_=d_vecs[3 * BLOC : 4 * BLOC, :])
    nc.sync.dma_start(out=sb_w1, in_=d_w1[:, :])
    nc.sync.dma_start(out=sb_w2, in_=d_w2.rearrange("(t p) n -> p t n", p=128))
    nc.sync.dma_start(out=sb_b1row, in_=d_b1[:, :])
    nc.sync.dma_start(out=sb_t, in_=d_t[:, :])

    # ---------------- constants ----------------
    ident = consts.tile([128, 128], F32)
    make_identity(nc, ident)
    ones_row = consts.tile([1, 128], F32)
    nc.vector.memset(ones_row, 1.0)
    ones_col = consts.tile([128, 1], F32)
    nc.vector.memset(ones_col, 1.0)
    ones2 = consts.tile([128, 2], F32)
    nc.vector.memset(ones2, 1.0)
    ones2_r = consts.tile([128, 2], F32R)
    nc.vector.tensor_copy(ones2_r, ones2)

    # rounded weights for f32r matmuls
    w1_r = consts.tile([128, H], F32R)
    nc.vector.tensor_copy(w1_r, sb_w1)

    # W2^T [j, m] and W1^T blocks [m, i]
    sb_w2t = consts.tile([128, H], F32)
    w2t_r = consts.tile([128, H], F32R)
    p_w2t = pbig.tile([128, 256], F32, tag="big")
    for t in range(2):
        nc.tensor.transpose(out=p_w2t[:, ts(t)], in_=sb_w2[:, t, :], identity=ident)
    nc.scalar.copy(out=sb_w2t, in_=p_w2t)
    nc.vector.tensor_copy(w2t_r, p_w2t)

    w2_r = consts.tile([128, 2, 128], F32R)
    nc.vector.tensor_copy(w2_r, sb_w2)

    sb_w1t = consts.tile([128, H], F32R)  # block t = (W1[:, t*128:+128])^T
    p_w1t = pbig.tile([128, 256], F32, tag="big")
    for t in range(2):
        nc.tensor.transpose(out=p_w1t[:, ts(t)], in_=sb_w1[:, ts(t)], identity=ident)
    nc.scalar.copy(out=sb_w1t, in_=p_w1t)

    # b1 in column form [m%128, m//128] straight from DRAM
    sb_b1c = consts.tile([128, 2], F32)
    nc.sync.dma_start(out=sb_b1c, in_=d_b1.rearrange("a (t p) -> p (a t)", p=128))
    # t replicated over 128 partitions via broadcast DMA
    t128 = consts.tile([128, 1], F32)
    nc.sync.dma_start(out=t128, in_=d_t[:, :].to_broadcast((128, 1)))

    # K = W1^T W1, L = W2 W2^T (f32r fast mode), KL = K.*L
    sb_K = consts.tile([128, 2, H], F32R)
    lv = consts.tile([128, 2, 2 * H], F32R)  # [:, qt, 0:256]=L, [:, qt, 256:512]=V_s
    sb_L = consts.tile([128, 2, H], F32)
    sb_KL = consts.tile([128, 2, H], F32)
    for t in range(2):
        p_k = pbig.tile([128, 256], F32, tag="big")
        nc.tensor.matmul(p_k, w1_r[:, ts(t)], w1_r, start=True, stop=True)
        nc.scalar.copy(out=sb_K[:, t, :], in_=p_k)
    for t in range(2):
        p_l = pbig.tile([128, 256], F32, tag="big")
        nc.tensor.matmul(p_l, w2t_r[:, ts(t)], w2t_r, start=True, stop=True)
        nc.scalar.copy(out=sb_L[:, t, :], in_=p_l)
        nc.vector.tensor_copy(lv[:, t, 0:H], p_l)
    for t in range(2):
        nc.vector.tensor_mul(sb_KL[:, t, :], sb_K[:, t, :], sb_L[:, t, :])

    # ---------------- t window: wf4 = 4t(1-t) ----------------
    omt = work.tile([128, 1], F32)
    nc.vector.tensor_scalar(
        out=omt, in0=t128, scalar1=-1.0, scalar2=1.0, op0=OP.mult, op1=OP.add
    )
    wf4 = work.tile([128, 1], F32)
    nc.vector.tensor_mul(wf4, omt, t128)                       # t*(1-t)
    nc.vector.tensor_scalar_mul(wf4, wf4, 4.0)                 # 4t(1-t)

    # ---------------- columns of x0/x1/dev/vel; x in column space ------
    p_vt = psmall.tile([128, 4 * BLOC], F32, tag="small")
    nc.tensor.transpose(
        out=p_vt, in_=sb_vecs, identity=ident[0 : 4 * BLOC, 0 : 4 * BLOC]
    )
    vc = work.tile([128, 4 * BLOC], F32R)  # cols: x0 | x1 | dev | vel
    nc.scalar.copy(out=vc, in_=p_vt)
    x0c, x1c = vc[:, 0:2], vc[:, 2:4]
    devc, velc = vc[:, 4:6], vc[:, 6:8]

    dxc = work.tile([128, BLOC], F32R)
    nc.vector.tensor_sub(dxc, x1c, x0c)
    xc = work.tile([128, BLOC], F32R)
    nc.vector.scalar_tensor_tensor(
        out=xc, in0=dxc, scalar=t128, in1=x0c, op0=OP.mult, op1=OP.add
    )
    nc.vector.scalar_tensor_tensor(
        out=xc, in0=devc, scalar=wf4, in1=xc, op0=OP.mult, op1=OP.add
    )

    # u columns per H-tile: [m%128, tile, sample]; h = tanh(u + b1)
    p_uc = psmall.tile([128, 2, BLOC], F32, tag="small")
    for t in range(2):
        nc.tensor.matmul(p_uc[:, t, :], w1_r[:, ts(t)], xc, start=True, stop=True)
    h_c = work.tile([128, 2, BLOC], F32)
    for t in range(2):
        nc.scalar.activation(
            out=h_c[:, t, :], in_=p_uc[:, t, :], func=AF.Tanh,
            bias=sb_b1c[:, t : t + 1], scale=1.0,
        )
    d_c = work.tile([128, 2, BLOC], F32)
    nc.vector.tensor_mul(d_c, h_c, h_c)
    nc.vector.tensor_scalar(
        out=d_c, in0=d_c, scalar1=-1.0, scalar2=1.0, op0=OP.mult, op1=OP.add
    )
    ehd_c = work.tile([128, 2, BLOC, 2], F32R)  # [...,0]=e, [...,1]=h*d
    nc.vector.tensor_mul(ehd_c[:, :, :, 1], h_c, d_c)
    nc.vector.tensor_scalar_mul(ehd_c[:, :, :, 0], ehd_c[:, :, :, 1], -2.0)
    e_c = ehd_c[:, :, :, 0]
    hd_c = ehd_c[:, :, :, 1]

    # ---------------- S1/S2 small matvecs (both samples batched) -------
    p_gw = psmall.tile([128, 2, 2, BLOC], F32, tag="small")  # [t, {g,w}, s]
    for t in range(2):
        nc.tensor.matmul(p_gw[:, t, 0, :], w2t_r[:, ts(t)], velc, start=True, stop=True)
        nc.tensor.matmul(p_gw[:, t, 1, :], w1_r[:, ts(t)], velc, start=True, stop=True)
    p_g = p_gw[:, :, 0, :]
    p_w = p_gw[:, :, 1, :]
    dgy = work.tile([128, 2, 2, BLOC], F32R)  # [qt, {dg,yv}, s]
    eg = work.tile([128, 2, BLOC], F32)
    nc.vector.tensor_mul(dgy[:, :, 0, :], p_g, d_c)          # d.*g
    nc.vector.tensor_mul(eg, p_g, e_c)                       # e.*g
    nc.vector.tensor_mul(dgy[:, :, 1, :], eg, p_w)           # e.*g.*w
    p_cvky = psmall.tile([128, 2, 2, BLOC], F32, tag="small")  # [mt, {cv,ky}, s]
    for mt in range(2):
        for qt in range(2):
            nc.tensor.matmul(
                p_cvky[:, mt, :, :], sb_K[:, qt, ts(mt)], dgy[:, qt, :, :],
                start=(qt == 0), stop=(qt == 1),
            )
    p_cv = p_cvky[:, :, 0, :]
    p_ky = p_cvky[:, :, 1, :]
    z2 = work.tile([128, 2, BLOC], F32R)
    nc.vector.tensor_mul(z2, eg, p_cv)            # e.*g.*cv
    i1 = work.tile([128, 2, BLOC], F32R)
    nc.vector.tensor_mul(i1, e_c, p_cv)           # e.*cv
    nc.vector.tensor_mul(i1, i1, p_w)             # e.*cv.*w
    i2 = work.tile([128, 2, BLOC], F32R)
    nc.vector.tensor_mul(i2, d_c, p_ky)           # d.*(K y)
    inner = work.tile([128, 2, BLOC], F32R)
    nc.vector.tensor_add(inner, i1, i2)

    # S1 rows -> psum[:, 0:128], 0.5*S2 rows -> psum[:, 128:256]
    p_s12 = prow.tile([BLOC, 256], F32, tag="rows")
    for qt in range(2):
        nc.tensor.matmul(
            p_s12[:, 0:128], inner[:, qt, :], w2_r[:, qt, :],
            start=(qt == 0), stop=(qt == 1),
        )
    for qt in range(2):
        nc.tensor.matmul(
            p_s12[:, 128:256], z2[:, qt, :], sb_w1t[:, ts(qt)],
            start=(qt == 0), stop=(qt == 1),
        )

    # ||v||^2 via ACT square-accumulate; restoration rows
    vsq = work.tile([BLOC, N], F32)
    svals = work.tile([BLOC, 2], F32)  # col0 = ||dG||^2, col1 = ||v||^2
    nc.scalar.activation(
        out=vsq, in_=vel_rows, func=AF.Square, accum_out=svals[:, 1:2]
    )
    rest = work.tile([BLOC, N], F32)
    nc.scalar.mul(out=rest, in_=dev_rows, mul=-0.1)

    # ---------------- per-sample norm path ----------------
    # nrm^2 = 8 * (h.*d)^T F (h.*d),  F = (K.*L).*U + K.*V.*W
    acc_cols = work.tile([128, BLOC], F32R)
    # Kd for both samples: [qt, s, 256] so V's rhs is [128, 512]
    kd2 = work.tile([128, 2, BLOC, H], F32R)
    for t in range(2):
        for s in range(BLOC):
            nc.scalar.activation(
                out=kd2[:, t, s, :], in_=sb_K[:, t, :], func=AF.Copy,
                scale=d_c[:, t, s : s + 1],
            )
    # V = L diag(d) K for both samples: psum [128, s*256 | s*256+256] per pt
    p_vs = []
    for pt in range(2):
        p_v = pbig.tile([128, 512], F32, tag="big")
        for qt in range(2):
            nc.tensor.matmul(
                p_v, lv[:, qt, ts(pt)], kd2[:, qt, :, :],
                start=(qt == 0), stop=(qt == 1),
            )
        p_vs.append(p_v)
    for s in range(BLOC):
        # V_s into lv's second half (rhs for W|U); vk = V_s .* K (gpsimd)
        vk = loop.tile([128, 2, H], F32, tag="vk")
        for qt in range(2):
            nc.scalar.copy(
                out=lv[:, qt, H : 2 * H], in_=p_vs[qt][:, s * H : (s + 1) * H]
            )
        for qt in range(2):
            nc.gpsimd.tensor_mul(
                vk[:, qt, :], lv.bitcast(F32)[:, qt, H : 2 * H], sb_K.bitcast(F32)[:, qt, :]
            )
        # [W | U] = Kd^T @ [L | V]: psum [128, 0:256]=W, [128, 256:512]=U
        F_r = loop.tile([128, 2, H], F32R, tag="F_r")
        for pt in range(2):
            p_wu = pbig.tile([128, 512], F32, tag="big")
            for qt in range(2):
                nc.tensor.matmul(
                    p_wu, kd2[:, qt, s, ts(pt)], lv[:, qt, :],
                    start=(qt == 0), stop=(qt == 1),
                )
            q_sb = loop.tile([128, H], F32, tag="q_sb")
            nc.vector.tensor_mul(q_sb, p_wu[:, H : 2 * H], sb_KL[:, pt, :])
            r_sb = loop.tile([128, H], F32, tag="r_sb")
            nc.vector.tensor_mul(r_sb, vk[:, pt, :], p_wu[:, 0:H])
            nc.gpsimd.tensor_add(F_r[:, pt, :], q_sb, r_sb)
        # matvec F (h.*d): rhs pair [e|hd], use hd column
        p_f = psmall.tile([128, 2, 2], F32, tag="small")  # [mt, {e,hd}]
        for mt in range(2):
            for qt in range(2):
                nc.tensor.matmul(
                    p_f[:, mt, :], F_r[:, qt, ts(mt)], ehd_c[:, qt, s, :],
                    start=(qt == 0), stop=(qt == 1),
                )
        scr = loop.tile([128, 2], F32, tag="scr")
        nc.vector.tensor_mul(scr, p_f[:, :, 1], hd_c[:, :, s])
        with nc.allow_low_precision("f32r accum rounding ~1e-6, fine here"):
            nc.vector.reduce_sum(acc_cols[:, s : s + 1], scr, axis=AX.X)

    # ---------------- final scalars & output ----------------
    p_sc = psmall.tile([BLOC, 2], F32, tag="small")
    nc.tensor.matmul(p_sc, acc_cols, ones2_r, start=True, stop=True)
    nc.scalar.mul(out=svals[:, 0:1], in_=p_sc[:, 0:1], mul=8.0)  # ||dG||^2

    # sqrt via bit trick + 2 Newton steps (avoids 2nd ACT table load)
    y = work.tile([BLOC, 2], F32)
    nc.vector.tensor_scalar(
        out=y.bitcast(I32), in0=svals.bitcast(I32),
        scalar1=1, scalar2=None, op0=OP.arith_shift_right,
    )
    nc.vector.tensor_scalar(
        out=y.bitcast(I32), in0=y.bitcast(I32),
        scalar1=SQRT_MAGIC, scalar2=None, op0=OP.add,
    )
    rcp = work.tile([BLOC, 2], F32)
    qn = work.tile([BLOC, 2], F32)
    for _ in range(2):
        nc.vector.reciprocal(rcp, y)
        nc.vector.tensor_mul(qn, svals, rcp)          # s / y
        nc.vector.tensor_add(y, y, qn)                # y + s/y
        nc.vector.tensor_scalar_mul(y, y, 0.5)        # 0.5*(y + s/y)
    nc.vector.tensor_scalar_add(y, y, 1e-6)
    den = work.tile([BLOC, 1], F32)
    nc.vector.tensor_mul(den, y[:, 0:1], y[:, 1:2])
    inv = work.tile([BLOC, 1], F32)
    nc.vector.reciprocal(inv, den)

    # a = (0.5*S2 - S1) * inv ; out_bot = a + (-0.1*dev)
    s2h = work.tile([BLOC, N], F32)
    nc.scalar.copy(out=s2h, in_=p_s12[:, 128:256])
    comb = work.tile([BLOC, N], F32)
    nc.vector.tensor_sub(comb, s2h, p_s12[:, 0:128])
    bot = work.tile([BLOC, N], F32)
    nc.vector.scalar_tensor_tensor(
        out=bot, in0=comb, scalar=inv, in1=rest, op0=OP.mult, op1=OP.add
    )
    nc.sync.dma_start(out=d_out[:, :], in_=bot)


_NC_CACHE = None


def _get_nc():
    global _NC_CACHE
    if _NC_CACHE is None:
        _NC_CACHE = build_nc()
    return _NC_CACHE


def make_in_maps(inputs):
    """Shard full inputs into per-core input maps."""
    state = np.ascontiguousarray(np.asarray(inputs["state_batch"], dtype=np.float32))
    x0 = np.asarray(inputs["x0_batch"], dtype=np.float32)
    x1 = np.asarray(inputs["x1_batch"], dtype=np.float32)
    W1 = np.ascontiguousarray(np.asarray(inputs["W1"], dtype=np.float32))
    W2 = np.ascontiguousarray(np.asarray(inputs["W2"], dtype=np.float32))
    b1 = np.ascontiguousarray(np.asarray(inputs["b1"], dtype=np.float32).reshape(1, H))
    t = np.ascontiguousarray(np.asarray(inputs["t"], dtype=np.float32).reshape(1, 1))
    dev, vel = state[:B], state[B:]
    in_maps = []
    for c in range(NCORES):
        sl = slice(c * BLOC, (c + 1) * BLOC)
        vecs = np.concatenate([x0[sl], x1[sl], dev[sl], vel[sl]], axis=0)
        in_maps.append(
            {
                "vecs": np.ascontiguousarray(vecs),
                "W1": W1,
                "W2": W2,
                "b1": b1,
                "t": t,
            }
        )
    return in_maps, vel


def kernel(**inputs) -> np.ndarray:
    from concourse.bass_utils import run_bass_kernel_spmd

    nc = _get_nc()
    in_maps, vel = make_in_maps(inputs)
    res = run_bass_kernel_spmd(nc, in_maps, core_ids=list(range(NCORES)))
    bottom = np.concatenate([res.results[c]["out_bot"] for c in range(NCORES)], axis=0)
    return np.concatenate([vel, bottom], axis=0).astype(np.float32)


# revision 22
# speedup vs baseline: 1.2626x; 1.0310x over previous
"""Trainium2 Bass kernel for BatchedModelManifoldGeodesicFlow.

Closed-form math (per sample), derived from the reference's autodiff:
  f(x) = tanh(x@W1 + b1)@W2 + b2,  J = jacrev(f)(x) = W2^T diag(d) W1^T
  with h = tanh(x@W1+b1), d = 1-h^2, e = -2*h*d, K = W1^T W1, L = W2 W2^T.
  V := L diag(d) K,  W := K diag(d) L (= V^T),  U := K diag(d) V,
  Q := (K.*L).*U,    R := K.*V.*W          (Q, R symmetric)
    ||dG||^2 = 2*( e^T Q e + 4 * (h.*d)^T R (h.*d) )
  Christoffel contraction -> small matvecs:
    S1 = W2^T [ e.*cv.*w + d.*(K (e.*g.*w)) ],  0.5*S2 = W1 (e.*g.*cv)
    w = W1^T v, g = W2 v, cv = K (d.*g)
    a = (0.5*S2 - S1) / ((||dG||+1e-6) * (||v||+1e-6))
  out = concat([v, a - 0.1*dev], axis=0)

Sharding: pure data parallel, batch 16 -> 2 samples per core on 8 cores.
Big [256x256x256] matmuls run as float32r (PE fast fp32 mode); their
operands are pre-rounded to f32r by the producing copy instructions.
"""

import sys

if "/opt/trn_rl_repo" not in sys.path:
    sys.path.insert(0, "/opt/trn_rl_repo")

import numpy as np

import concourse.bacc as bacc
import concourse.tile as tile
from concourse import mybir
from concourse.masks import make_identity

N = 128
H = 256
B = 16
NCORES = 8
BLOC = B // NCORES  # 2 samples per core

F32 = mybir.dt.float32
F32R = mybir.dt.float32r
BF16 = mybir.dt.bfloat16
I32 = mybir.dt.int32
AF = mybir.ActivationFunctionType
OP = mybir.AluOpType
AX = mybir.AxisListType

SQRT_MAGIC = 0x1FBD1DF5  # bits trick: sqrt(x) ~ bitcast((bits(x)>>1) + MAGIC)


def ts(i, sz=128):
    return slice(i * sz, (i + 1) * sz)


def build_nc():
    nc = bacc.Bacc(trn_type="TRN2", enable_partition_id=False)

    d_vecs = nc.dram_tensor("vecs", [4 * BLOC, N], F32, kind="ExternalInput")
    d_w1 = nc.dram_tensor("W1", [N, H], F32, kind="ExternalInput")
    d_w2 = nc.dram_tensor("W2", [H, N], F32, kind="ExternalInput")
    d_b1 = nc.dram_tensor("b1", [1, H], F32, kind="ExternalInput")
    d_t = nc.dram_tensor("t", [1, 1], F32, kind="ExternalInput")
    d_out = nc.dram_tensor("out_bot", [BLOC, N], F32, kind="ExternalOutput")

    with tile.TileContext(nc) as tc:
        with (
            tc.tile_pool(name="consts", bufs=1) as consts,
            tc.tile_pool(name="work", bufs=1) as work,
            tc.tile_pool(name="loop", bufs=2) as loop,
            tc.tile_pool(name="pbig", bufs=4, space="PSUM") as pbig,
            tc.tile_pool(name="psmall", bufs=3, space="PSUM") as psmall,
            tc.tile_pool(name="prow", bufs=1, space="PSUM") as prow,
        ):
            _emit(nc, consts, work, loop, pbig, psmall, prow,
                  d_vecs, d_w1, d_w2, d_b1, d_t, d_out)

    nc.compile()  # Bacc: split multi-waits into event semaphores, alloc regs
    return nc


def _emit(nc, consts, work, loop, pbig, psmall, prow,
          d_vecs, d_w1, d_w2, d_b1, d_t, d_out):
    # ---------------- input DMA ----------------
    sb_vecs = consts.tile([4 * BLOC, N], F32)   # rows: x0, x1, dev, vel
    dev_rows = consts.tile([BLOC, N], F32)
    vel_rows = consts.tile([BLOC, N], F32)
    sb_w1 = consts.tile([128, H], F32)          # W1[k, m]
    sb_w2 = consts.tile([128, 2, 128], F32)     # W2 tiles: [m%128, m//128, j]
    sb_b1row = consts.tile([1, H], F32)
    sb_t = consts.tile([1, 1], F32)
    nc.sync.dma_start(out=sb_vecs, in_=d_vecs[:, :])
    nc.sync.dma_start(out=dev_rows, in_=d_vecs[2 * BLOC : 3 * BLOC, :])
    nc.sync.dma_start(out=vel_rows, in_=d_vecs[3 * BLOC : 4 * BLOC, :])
    nc.sync.dma_start(out=sb_w1, in_=d_w1[:, :])
    nc.sync.dma_start(out=sb_w2, in_=d_w2.rearrange("(t p) n -> p t n", p=128))
    nc.sync.dma_start(out=sb_b1row, in_=d_b1[:, :])
    nc.sync.dma_start(out=sb_t, in_=d_t[:, :])

    # ---------------- constants ----------------
    ident = consts.tile([128, 128], F32)
    make_identity(nc, ident)
    ones_row = consts.tile([1, 128], F32)
    nc.vector.memset(ones_row, 1.0)
    ones_col = consts.tile([128, 1], F32)
    nc.vector.memset(ones_col, 1.0)
    ones2 = consts.tile([128, 2], F32)
    nc.vector.memset(ones2, 1.0)
    ones2_r = consts.tile([128, 2], F32R)
    nc.vector.tensor_copy(ones2_r, ones2)

    # rounded weights for f32r matmuls
    w1_r = consts.tile([128, H], F32R)
    nc.vector.tensor_copy(w1_r, sb_w1)

    # W2^T [j, m] and W1^T blocks [m, i]
    sb_w2t = consts.tile([128, H], F32)
    w2t_r = consts.tile([128, H], F32R)
    p_w2t = pbig.tile([128, 256], F32, tag="big")
    for t in range(2):
        nc.tensor.transpose(out=p_w2t[:, ts(t)], in_=sb_w2[:, t, :], identity=ident)
    nc.scalar.copy(out=sb_w2t, in_=p_w2t)
    nc.vector.tensor_copy(w2t_r, p_w2t)

    w2_r = consts.tile([128, 2, 128], F32R)
    nc.vector.tensor_copy(w2_r, sb_w2)

    sb_w1t = consts.tile([128, H], F32R)  # block t = (W1[:, t*128:+128])^T
    p_w1t = pbig.tile([128, 256], F32, tag="big")
    for t in range(2):
        nc.tensor.transpose(out=p_w1t[:, ts(t)], in_=sb_w1[:, ts(t)], identity=ident)
    nc.scalar.copy(out=sb_w1t, in_=p_w1t)

    # b1 in column form [m%128, m//128] straight from DRAM
    sb_b1c = consts.tile([128, 2], F32)
    nc.sync.dma_start(out=sb_b1c, in_=d_b1.rearrange("a (t p) -> p (a t)", p=128))
    # t replicated over 128 partitions via broadcast DMA
    t128 = consts.tile([128, 1], F32)
    nc.sync.dma_start(out=t128, in_=d_t[:, :].to_broadcast((128, 1)))

    # K = W1^T W1, L = W2 W2^T (f32r fast mode), KL = K.*L
    sb_K = consts.tile([128, 2, H], F32R)
    kb = consts.tile([128, 2, H], BF16)
    klb = consts.tile([128, 2, H], BF16)
    lv = consts.tile([128, 2, 2 * H], BF16)  # [:, qt, 0:256]=L, [:, qt, 256:512]=V_s
    sb_L = consts.tile([128, 2, H], F32)
    sb_KL = consts.tile([128, 2, H], F32)
    for t in range(2):
        p_k = pbig.tile([128, 256], F32, tag="big")
        nc.tensor.matmul(p_k, w1_r[:, ts(t)], w1_r, start=True, stop=True)
        nc.scalar.copy(out=sb_K[:, t, :], in_=p_k)
        nc.vector.tensor_copy(kb[:, t, :], sb_K.bitcast(F32)[:, t, :])
    for t in range(2):
        p_l = pbig.tile([128, 256], F32, tag="big")
        nc.tensor.matmul(p_l, w2t_r[:, ts(t)], w2t_r, start=True, stop=True)
        nc.scalar.copy(out=sb_L[:, t, :], in_=p_l)
        nc.vector.tensor_copy(lv[:, t, 0:H], p_l)
    for t in range(2):
        nc.vector.tensor_mul(sb_KL[:, t, :], sb_K[:, t, :], sb_L[:, t, :])
        nc.vector.tensor_copy(klb[:, t, :], sb_KL[:, t, :])

    # ---------------- t window: wf4 = 4t(1-t) ----------------
    omt = work.tile([128, 1], F32)
    nc.vector.tensor_scalar(
        out=omt, in0=t128, scalar1=-1.0, scalar2=1.0, op0=OP.mult, op1=OP.add
    )
    wf4 = work.tile([128, 1], F32)
    nc.vector.tensor_mul(wf4, omt, t128)                       # t*(1-t)
    nc.vector.tensor_scalar_mul(wf4, wf4, 4.0)                 # 4t(1-t)

    # ---------------- columns of x0/x1/dev/vel; x in column space ------
    p_vt = psmall.tile([128, 4 * BLOC], F32, tag="small")
    nc.tensor.transpose(
        out=p_vt, in_=sb_vecs, identity=ident[0 : 4 * BLOC, 0 : 4 * BLOC]
    )
    vc = work.tile([128, 4 * BLOC], F32R)  # cols: x0 | x1 | dev | vel
    nc.scalar.copy(out=vc, in_=p_vt)
    x0c, x1c = vc[:, 0:2], vc[:, 2:4]
    devc, velc = vc[:, 4:6], vc[:, 6:8]

    dxc = work.tile([128, BLOC], F32R)
    nc.vector.tensor_sub(dxc, x1c, x0c)
    xc = work.tile([128, BLOC], F32R)
    nc.vector.scalar_tensor_tensor(
        out=xc, in0=dxc, scalar=t128, in1=x0c, op0=OP.mult, op1=OP.add
    )
    nc.vector.scalar_tensor_tensor(
        out=xc, in0=devc, scalar=wf4, in1=xc, op0=OP.mult, op1=OP.add
    )

    # u columns per H-tile: [m%128, tile, sample]; h = tanh(u + b1)
    p_uc = psmall.tile([128, 2, BLOC], F32, tag="small")
    for t in range(2):
        nc.tensor.matmul(p_uc[:, t, :], w1_r[:, ts(t)], xc, start=True, stop=True)
    h_c = work.tile([128, 2, BLOC], F32)
    for t in range(2):
        nc.scalar.activation(
            out=h_c[:, t, :], in_=p_uc[:, t, :], func=AF.Tanh,
            bias=sb_b1c[:, t : t + 1], scale=1.0,
        )
    d_c = work.tile([128, 2, BLOC], F32)
    nc.vector.tensor_mul(d_c, h_c, h_c)
    nc.vector.tensor_scalar(
        out=d_c, in0=d_c, scalar1=-1.0, scalar2=1.0, op0=OP.mult, op1=OP.add
    )
    ehd_c = work.tile([128, 2, BLOC, 2], F32R)  # [...,0]=e, [...,1]=h*d
    nc.vector.tensor_mul(ehd_c[:, :, :, 1], h_c, d_c)
    nc.vector.tensor_scalar_mul(ehd_c[:, :, :, 0], ehd_c[:, :, :, 1], -2.0)
    e_c = ehd_c[:, :, :, 0]
    hd_c = ehd_c[:, :, :, 1]
    ehd_b = work.tile([128, 2, BLOC, 2], BF16)
    nc.vector.tensor_copy(ehd_b, ehd_c.bitcast(F32))

    # ---------------- S1/S2 small matvecs (both samples batched) -------
    p_gw = psmall.tile([128, 2, 2, BLOC], F32, tag="small")  # [t, {g,w}, s]
    for t in range(2):
        nc.tensor.matmul(p_gw[:, t, 0, :], w2t_r[:, ts(t)], velc, start=True, stop=True)
        nc.tensor.matmul(p_gw[:, t, 1, :], w1_r[:, ts(t)], velc, start=True, stop=True)
    p_g = p_gw[:, :, 0, :]
    p_w = p_gw[:, :, 1, :]
    dgy = work.tile([128, 2, 2, BLOC], F32R)  # [qt, {dg,yv}, s]
    eg = work.tile([128, 2, BLOC], F32)
    nc.vector.tensor_mul(dgy[:, :, 0, :], p_g, d_c)          # d.*g
    nc.vector.tensor_mul(eg, p_g, e_c)                       # e.*g
    nc.vector.tensor_mul(dgy[:, :, 1, :], eg, p_w)           # e.*g.*w
    p_cvky = psmall.tile([128, 2, 2, BLOC], F32, tag="small")  # [mt, {cv,ky}, s]
    for mt in range(2):
        for qt in range(2):
            nc.tensor.matmul(
                p_cvky[:, mt, :, :], sb_K[:, qt, ts(mt)], dgy[:, qt, :, :],
                start=(qt == 0), stop=(qt == 1),
            )
    p_cv = p_cvky[:, :, 0, :]
    p_ky = p_cvky[:, :, 1, :]
    z2 = work.tile([128, 2, BLOC], F32R)
    nc.vector.tensor_mul(z2, eg, p_cv)            # e.*g.*cv
    i1 = work.tile([128, 2, BLOC], F32R)
    nc.vector.tensor_mul(i1, e_c, p_cv)           # e.*cv
    nc.vector.tensor_mul(i1, i1, p_w)             # e.*cv.*w
    i2 = work.tile([128, 2, BLOC], F32R)
    nc.vector.tensor_mul(i2, d_c, p_ky)           # d.*(K y)
    inner = work.tile([128, 2, BLOC], F32R)
    nc.vector.tensor_add(inner, i1, i2)

    # S1 rows -> psum[:, 0:128], 0.5*S2 rows -> psum[:, 128:256]
    p_s12 = prow.tile([BLOC, 256], F32, tag="rows")
    for qt in range(2):
        nc.tensor.matmul(
            p_s12[:, 0:128], inner[:, qt, :], w2_r[:, qt, :],
            start=(qt == 0), stop=(qt == 1),
        )
    for qt in range(2):
        nc.tensor.matmul(
            p_s12[:, 128:256], z2[:, qt, :], sb_w1t[:, ts(qt)],
            start=(qt == 0), stop=(qt == 1),
        )

    # ||v||^2 via ACT square-accumulate; restoration rows
    vsq = work.tile([BLOC, N], F32)
    svals = work.tile([BLOC, 2], F32)  # col0 = ||dG||^2, col1 = ||v||^2
    nc.scalar.activation(
        out=vsq, in_=vel_rows, func=AF.Square, accum_out=svals[:, 1:2]
    )
    rest = work.tile([BLOC, N], F32)
    nc.scalar.mul(out=rest, in_=dev_rows, mul=-0.1)

    # ---------------- per-sample norm path (bf16 matmuls) ----------------
    # nrm^2 = 8 * (h.*d)^T F (h.*d),  F = (K.*L).*U + K.*V.*W
    acc_cols = work.tile([128, BLOC], F32R)
    # Kd for both samples: [qt, s, 256] so V's rhs is [128, 512]
    kd2 = work.tile([128, 2, BLOC, H], BF16)
    for t in range(2):
        for s in range(BLOC):
            nc.vector.tensor_scalar_mul(
                kd2[:, t, s, :], sb_K.bitcast(F32)[:, t, :], d_c[:, t, s : s + 1]
            )
    # V = L diag(d) K for both samples: psum [128, s*256 | s*256+256] per pt
    p_vs = []
    for pt in range(2):
        p_v = pbig.tile([128, 512], F32, tag="big")
        for qt in range(2):
            nc.tensor.matmul(
                p_v, lv[:, qt, ts(pt)], kd2[:, qt, :, :],
                start=(qt == 0), stop=(qt == 1),
            )
        p_vs.append(p_v)
    for s in range(BLOC):
        # V_s into lv's second half (rhs for W|U); vk = V_s .* K (gpsimd)
        vk = loop.tile([128, 2, H], BF16, tag="vk")
        for qt in range(2):
            nc.vector.tensor_copy(
                lv[:, qt, H : 2 * H], p_vs[qt][:, s * H : (s + 1) * H]
            )
        for qt in range(2):
            nc.gpsimd.tensor_mul(vk[:, qt, :], lv[:, qt, H : 2 * H], kb[:, qt, :])
        # [W | U] = Kd^T @ [L | V]: psum [128, 0:256]=W, [128, 256:512]=U
        F_b = loop.tile([128, 2, H], BF16, tag="F_b")
        for pt in range(2):
            p_wu = pbig.tile([128, 512], F32, tag="big")
            for qt in range(2):
                nc.tensor.matmul(
                    p_wu, kd2[:, qt, s, ts(pt)], lv[:, qt, :],
                    start=(qt == 0), stop=(qt == 1),
                )
            q_sb = loop.tile([128, H], BF16, tag="q_sb")
            nc.vector.tensor_mul(q_sb, p_wu[:, H : 2 * H], klb[:, pt, :])
            r_sb = loop.tile([128, H], BF16, tag="r_sb")
            nc.vector.tensor_mul(r_sb, vk[:, pt, :], p_wu[:, 0:H])
            nc.gpsimd.tensor_add(F_b[:, pt, :], q_sb, r_sb)
        # matvec F (h.*d): rhs pair [e|hd], use hd column
        p_f = psmall.tile([128, 2, 2], F32, tag="small")  # [mt, {e,hd}]
        for mt in range(2):
            for qt in range(2):
                nc.tensor.matmul(
                    p_f[:, mt, :], F_b[:, qt, ts(mt)], ehd_b[:, qt, s, :],
                    start=(qt == 0), stop=(qt == 1),
                )
        scr = loop.tile([128, 2], F32, tag="scr")
        nc.vector.tensor_mul(scr, p_f[:, :, 1], hd_c[:, :, s])
        with nc.allow_low_precision("f32r accum rounding ~1e-6, fine here"):
            nc.vector.reduce_sum(acc_cols[:, s : s + 1], scr, axis=AX.X)

    # ---------------- final scalars & output ----------------
    p_sc = psmall.tile([BLOC, 2], F32, tag="small")
    nc.tensor.matmul(p_sc, acc_cols, ones2_r, start=True, stop=True)
    nc.scalar.mul(out=svals[:, 0:1], in_=p_sc[:, 0:1], mul=8.0)  # ||dG||^2

    # sqrt via bit trick + 2 Newton steps (avoids 2nd ACT table load)
    y = work.tile([BLOC, 2], F32)
    nc.vector.tensor_scalar(
        out=y.bitcast(I32), in0=svals.bitcast(I32),
        scalar1=1, scalar2=None, op0=OP.arith_shift_right,
    )
    nc.vector.tensor_scalar(
        out=y.bitcast(I32), in0=y.bitcast(I32),
        scalar1=SQRT_MAGIC, scalar2=None, op0=OP.add,
    )
    rcp = work.tile([BLOC, 2], F32)
    qn = work.tile([BLOC, 2], F32)
    for _ in range(1):
        nc.vector.reciprocal(rcp, y)
        nc.vector.tensor_mul(qn, svals, rcp)          # s / y
        nc.vector.tensor_add(y, y, qn)                # y + s/y
        nc.vector.tensor_scalar_mul(y, y, 0.5)        # 0.5*(y + s/y)
    nc.vector.tensor_scalar_add(y, y, 1e-6)
    den = work.tile([BLOC, 1], F32)
    nc.vector.tensor_mul(den, y[:, 0:1], y[:, 1:2])
    inv = work.tile([BLOC, 1], F32)
    nc.vector.reciprocal(inv, den)

    # a = (0.5*S2 - S1) * inv ; out_bot = a + (-0.1*dev)
    s2h = work.tile([BLOC, N], F32)
    nc.scalar.copy(out=s2h, in_=p_s12[:, 128:256])
    comb = work.tile([BLOC, N], F32)
    nc.vector.tensor_sub(comb, s2h, p_s12[:, 0:128])
    bot = work.tile([BLOC, N], F32)
    nc.vector.scalar_tensor_tensor(
        out=bot, in0=comb, scalar=inv, in1=rest, op0=OP.mult, op1=OP.add
    )
    nc.sync.dma_start(out=d_out[:, :], in_=bot)


_NC_CACHE = None


def _get_nc():
    global _NC_CACHE
    if _NC_CACHE is None:
        _NC_CACHE = build_nc()
    return _NC_CACHE


def make_in_maps(inputs):
    """Shard full inputs into per-core input maps."""
    state = np.ascontiguousarray(np.asarray(inputs["state_batch"], dtype=np.float32))
    x0 = np.asarray(inputs["x0_batch"], dtype=np.float32)
    x1 = np.asarray(inputs["x1_batch"], dtype=np.float32)
    W1 = np.ascontiguousarray(np.asarray(inputs["W1"], dtype=np.float32))
    W2 = np.ascontiguousarray(np.asarray(inputs["W2"], dtype=np.float32))
    b1 = np.ascontiguousarray(np.asarray(inputs["b1"], dtype=np.float32).reshape(1, H))
    t = np.ascontiguousarray(np.asarray(inputs["t"], dtype=np.float32).reshape(1, 1))
    dev, vel = state[:B], state[B:]
    in_maps = []
    for c in range(NCORES):
        sl = slice(c * BLOC, (c + 1) * BLOC)
        vecs = np.concatenate([x0[sl], x1[sl], dev[sl], vel[sl]], axis=0)
        in_maps.append(
            {
                "vecs": np.ascontiguousarray(vecs),
                "W1": W1,
                "W2": W2,
                "b1": b1,
                "t": t,
            }
        )
    return in_maps, vel


def kernel(**inputs) -> np.ndarray:
    from concourse.bass_utils import run_bass_kernel_spmd

    nc = _get_nc()
    in_maps, vel = make_in_maps(inputs)
    res = run_bass_kernel_spmd(nc, in_maps, core_ids=list(range(NCORES)))
    bottom = np.concatenate([res.results[c]["out_bot"] for c in range(NCORES)], axis=0)
    return np.concatenate([vel, bottom], axis=0).astype(np.float32)


# revision 26
# speedup vs baseline: 1.5605x; 1.2359x over previous
"""Trainium2 Bass kernel for BatchedModelManifoldGeodesicFlow.

Closed-form math (per sample), derived from the reference's autodiff:
  f(x) = tanh(x@W1 + b1)@W2 + b2,  J = jacrev(f)(x) = W2^T diag(d) W1^T
  with h = tanh(x@W1+b1), d = 1-h^2, e = -2*h*d, K = W1^T W1, L = W2 W2^T.
  V := L diag(d) K,  W := K diag(d) L (= V^T),  U := K diag(d) V,
  F := (K.*L).*U + K.*V.*W  (symmetric; uses e = -2*h*d)
    ||dG||^2 = 8 * (h.*d)^T F (h.*d)
  Christoffel contraction -> small matvecs:
    S1 = W2^T [ e.*cv.*w + d.*(K (e.*g.*w)) ],  0.5*S2 = W1 (e.*g.*cv)
    w = W1^T v, g = W2 v, cv = K (d.*g)
    a = (0.5*S2 - S1) / ((||dG||+1e-6) * (||v||+1e-6))
  out = concat([v, a - 0.1*dev], axis=0)

Sharding: pure data parallel, batch 16 -> 2 samples per core on 8 cores.
All heavy matmuls in bf16 (errors cancel statistically in the big norm
sums); the tanh-input matmul stays float32r for accuracy of h.
"""

import sys

if "/opt/trn_rl_repo" not in sys.path:
    sys.path.insert(0, "/opt/trn_rl_repo")

import numpy as np

import concourse.bacc as bacc
import concourse.tile as tile
from concourse import mybir
from concourse.masks import make_identity

N = 128
H = 256
B = 16
NCORES = 8
BLOC = B // NCORES  # 2 samples per core

F32 = mybir.dt.float32
F32R = mybir.dt.float32r
BF16 = mybir.dt.bfloat16
I32 = mybir.dt.int32
AF = mybir.ActivationFunctionType
OP = mybir.AluOpType
AX = mybir.AxisListType

SQRT_MAGIC = 0x1FBD1DF5  # bits trick: sqrt(x) ~ bitcast((bits(x)>>1) + MAGIC)


def ts(i, sz=128):
    return slice(i * sz, (i + 1) * sz)


def build_nc():
    nc = bacc.Bacc(trn_type="TRN2", enable_partition_id=False)

    # vecs rows: dev(0:2) x0(2:4) x1(4:6) vel(6:8)
    d_vecs = nc.dram_tensor("vecs", [4 * BLOC, N], F32, kind="ExternalInput")
    d_w1 = nc.dram_tensor("W1", [N, H], F32, kind="ExternalInput")
    d_w2 = nc.dram_tensor("W2", [H, N], F32, kind="ExternalInput")
    # misc cols: t128 | b1c (2 cols) | pad
    d_misc = nc.dram_tensor("misc", [128, 4], F32, kind="ExternalInput")
    d_out = nc.dram_tensor("out_bot", [BLOC, N], F32, kind="ExternalOutput")

    with tile.TileContext(nc) as tc:
        with (
            tc.tile_pool(name="consts", bufs=1) as consts,
            tc.tile_pool(name="work", bufs=1) as work,
            tc.tile_pool(name="loop", bufs=2) as loop,
            tc.tile_pool(name="pbig", bufs=4, space="PSUM") as pbig,
            tc.tile_pool(name="psmall", bufs=3, space="PSUM") as psmall,
            tc.tile_pool(name="prow", bufs=1, space="PSUM") as prow,
        ):
            _emit(nc, consts, work, loop, pbig, psmall, prow,
                  d_vecs, d_w1, d_w2, d_misc, d_out)

    nc.compile()  # Bacc: split multi-waits into event semaphores, alloc regs
    return nc


def _emit(nc, consts, work, loop, pbig, psmall, prow,
          d_vecs, d_w1, d_w2, d_misc, d_out):
    # ---------------- input DMA (2 trigger engines in parallel) --------
    sb_w1 = consts.tile([128, H], F32)          # W1[k, m]
    sb_w2 = consts.tile([128, 2, 128], F32)     # W2 tiles: [m%128, m//128, j]
    sb_vecs = consts.tile([4 * BLOC, N], F32)
    sb_misc = consts.tile([128, 4], F32)
    nc.sync.dma_start(out=sb_vecs, in_=d_vecs[:, :])
    nc.sync.dma_start(out=sb_w1, in_=d_w1[:, :])
    nc.sync.dma_start(out=sb_w2, in_=d_w2.rearrange("(t p) n -> p t n", p=128))
    nc.scalar.dma_start(out=sb_misc, in_=d_misc[:, :])
    t128 = sb_misc[:, 0:1]
    sb_b1c = sb_misc[:, 1:3]
    dev_rows = sb_vecs[0:BLOC, :]

    # ---------------- constants ----------------
    ident = consts.tile([128, 128], F32)
    make_identity(nc, ident)
    ones2 = consts.tile([128, 2], F32)
    nc.vector.memset(ones2, 1.0)
    ones2_r = consts.tile([128, 2], F32R)
    nc.vector.tensor_copy(ones2_r, ones2)

    # ---------------- t window: wf4 = 4t(1-t) ----------------
    omt = work.tile([128, 1], F32)
    nc.vector.tensor_scalar(
        out=omt, in0=t128, scalar1=-1.0, scalar2=1.0, op0=OP.mult, op1=OP.add
    )
    wf4 = work.tile([128, 1], F32)
    nc.vector.tensor_mul(wf4, omt, t128)                       # t*(1-t)
    nc.vector.tensor_scalar_mul(wf4, wf4, 4.0)                 # 4t(1-t)

    # ---------------- columns of dev/x0/x1/vel; x in column space ------
    p_vt = psmall.tile([128, 4 * BLOC], F32, tag="small")
    nc.tensor.transpose(
        out=p_vt, in_=sb_vecs, identity=ident[0 : 4 * BLOC, 0 : 4 * BLOC]
    )
    vc = work.tile([128, 4 * BLOC], F32R)  # cols: dev | x0 | x1 | vel
    nc.scalar.copy(out=vc, in_=p_vt)
    devc, x0c = vc[:, 0:2], vc[:, 2:4]
    x1c, velc = vc[:, 4:6], vc[:, 6:8]
    vc_b = work.tile([128, 4 * BLOC], BF16)
    nc.vector.tensor_copy(vc_b, vc.bitcast(F32))
    velc_b = vc_b[:, 6:8]

    # ||v||^2 via Gram matrix + diagonal extraction
    p_vv = psmall.tile([BLOC, BLOC], F32, tag="small")
    nc.tensor.matmul(p_vv, velc, velc, start=True, stop=True)
    svals = work.tile([BLOC, 2], F32)  # col0 = ||dG||^2, col1 = ||v||^2
    vvd = work.tile([BLOC, BLOC], F32)
    nc.vector.tensor_mul(vvd, p_vv, ident[0:BLOC, 0:BLOC])
    nc.vector.reduce_sum(svals[:, 1:2], vvd, axis=AX.X)

    dxc = work.tile([128, BLOC], F32R)
    nc.vector.tensor_sub(dxc, x1c, x0c)
    xc = work.tile([128, BLOC], F32R)
    nc.vector.scalar_tensor_tensor(
        out=xc, in0=dxc, scalar=t128, in1=x0c, op0=OP.mult, op1=OP.add
    )
    nc.vector.scalar_tensor_tensor(
        out=xc, in0=devc, scalar=wf4, in1=xc, op0=OP.mult, op1=OP.add
    )

    # weights: f32r for the tanh-input matmul, bf16 for everything else
    w1_r = consts.tile([128, H], F32R)
    nc.vector.tensor_copy(w1_r, sb_w1)
    w1_b = consts.tile([128, H], BF16)
    nc.vector.tensor_copy(w1_b, sb_w1)
    w2_b = consts.tile([128, 2, 128], BF16)
    nc.vector.tensor_copy(w2_b, sb_w2)

    # W2^T and W1^T blocks
    w2t_b = consts.tile([128, H], BF16)
    p_w2t = pbig.tile([128, 512], F32, tag="big")
    for t in range(2):
        nc.tensor.transpose(out=p_w2t[:, ts(t)], in_=sb_w2[:, t, :], identity=ident)
    nc.vector.tensor_copy(w2t_b, p_w2t[:, 0:256])

    w1t_b = consts.tile([128, H], BF16)  # block t = (W1[:, t*128:+128])^T
    p_w1t = pbig.tile([128, 512], F32, tag="big")
    for t in range(2):
        nc.tensor.transpose(out=p_w1t[:, ts(t)], in_=sb_w1[:, ts(t)], identity=ident)
    nc.vector.tensor_copy(w1t_b, p_w1t[:, 0:256])

    # K = W1^T W1, L = W2 W2^T (bf16), KL = K.*L
    kb = consts.tile([128, 2, H], BF16)
    klb = consts.tile([128, 2, H], BF16)
    lb = consts.tile([128, 2, H], BF16)
    for t in range(2):
        p_k = pbig.tile([128, 512], F32, tag="big")
        nc.tensor.matmul(p_k[:, 0:256], w1_b[:, ts(t)], w1_b, start=True, stop=True)
        nc.scalar.copy(out=kb[:, t, :], in_=p_k[:, 0:256])
    for t in range(2):
        p_l = pbig.tile([128, 512], F32, tag="big")
        nc.tensor.matmul(p_l[:, 0:256], w2t_b[:, ts(t)], w2t_b, start=True, stop=True)
        nc.vector.tensor_copy(lb[:, t, :], p_l[:, 0:256])
    for t in range(2):
        nc.vector.tensor_mul(klb[:, t, :], kb[:, t, :], lb[:, t, :])

    # u columns per H-tile: [m%128, tile, sample]; h = tanh(u + b1)
    p_uc = psmall.tile([128, 2, BLOC], F32, tag="small")
    for t in range(2):
        nc.tensor.matmul(p_uc[:, t, :], w1_r[:, ts(t)], xc, start=True, stop=True)
    h_c = work.tile([128, 2, BLOC], F32)
    for t in range(2):
        nc.scalar.activation(
            out=h_c[:, t, :], in_=p_uc[:, t, :], func=AF.Tanh,
            bias=sb_b1c[:, t : t + 1], scale=1.0,
        )
    d_c = work.tile([128, 2, BLOC], F32)
    nc.vector.tensor_mul(d_c, h_c, h_c)
    nc.vector.tensor_scalar(
        out=d_c, in0=d_c, scalar1=-1.0, scalar2=1.0, op0=OP.mult, op1=OP.add
    )
    ehd_c = work.tile([128, 2, BLOC, 2], F32R)  # [...,0]=e, [...,1]=h*d
    nc.vector.tensor_mul(ehd_c[:, :, :, 1], h_c, d_c)
    nc.vector.tensor_scalar_mul(ehd_c[:, :, :, 0], ehd_c[:, :, :, 1], -2.0)
    e_c = ehd_c[:, :, :, 0]
    hd_c = ehd_c[:, :, :, 1]
    ehd_b = work.tile([128, 2, BLOC, 2], BF16)
    nc.vector.tensor_copy(ehd_b, ehd_c.bitcast(F32))

    # ---------------- S1/S2 small matvecs (both samples batched) -------
    p_gw = psmall.tile([128, 2, 2, BLOC], F32, tag="small")  # [t, {g,w}, s]
    for t in range(2):
        nc.tensor.matmul(p_gw[:, t, 0, :], w2t_b[:, ts(t)], velc_b, start=True, stop=True)
        nc.tensor.matmul(p_gw[:, t, 1, :], w1_b[:, ts(t)], velc_b, start=True, stop=True)
    p_g = p_gw[:, :, 0, :]
    p_w = p_gw[:, :, 1, :]
    dgy = work.tile([128, 2, 2, BLOC], BF16)  # [qt, {dg,yv}, s]
    eg = work.tile([128, 2, BLOC], F32)
    nc.vector.tensor_mul(dgy[:, :, 0, :], p_g, d_c)          # d.*g
    nc.vector.tensor_mul(eg, p_g, e_c)                       # e.*g
    nc.vector.tensor_mul(dgy[:, :, 1, :], eg, p_w)           # e.*g.*w
    p_cvky = psmall.tile([128, 2, 2, BLOC], F32, tag="small")  # [mt, {cv,ky}, s]
    for mt in range(2):
        for qt in range(2):
            nc.tensor.matmul(
                p_cvky[:, mt, :, :], kb[:, qt, ts(mt)], dgy[:, qt, :, :],
                start=(qt == 0), stop=(qt == 1),
            )
    p_cv = p_cvky[:, :, 0, :]
    p_ky = p_cvky[:, :, 1, :]
    z2 = work.tile([128, 2, BLOC], BF16)
    nc.vector.tensor_mul(z2, eg, p_cv)            # e.*g.*cv
    i1 = work.tile([128, 2, BLOC], F32)
    nc.vector.tensor_mul(i1, e_c, p_cv)           # e.*cv
    nc.vector.tensor_mul(i1, i1, p_w)             # e.*cv.*w
    i2 = work.tile([128, 2, BLOC], F32)
    nc.vector.tensor_mul(i2, d_c, p_ky)           # d.*(K y)
    inner = work.tile([128, 2, BLOC], BF16)
    nc.vector.tensor_add(inner, i1, i2)

    # S1 rows -> psum[:, 0:128], 0.5*S2 rows -> psum[:, 128:256]
    p_s12 = prow.tile([BLOC, 256], F32, tag="rows")
    for qt in range(2):
        nc.tensor.matmul(
            p_s12[:, 0:128], inner[:, qt, :], w2_b[:, qt, :],
            start=(qt == 0), stop=(qt == 1),
        )
    for qt in range(2):
        nc.tensor.matmul(
            p_s12[:, 128:256], z2[:, qt, :], w1t_b[:, ts(qt)],
            start=(qt == 0), stop=(qt == 1),
        )

    rest = work.tile([BLOC, N], F32)
    nc.scalar.mul(out=rest, in_=dev_rows, mul=-0.1)

    # ---------------- per-sample norm path (bf16 matmuls) --------------
    # nrm^2 = 8 * (h.*d)^T F (h.*d),  F = (K.*L).*U + K.*V.*W
    acc_cols = work.tile([128, BLOC], F32R)
    # Kd for both samples: [qt, s, 256] so V's rhs is [128, 512]
    kd2 = work.tile([128, 2, BLOC, H], BF16)
    for t in range(2):
        for s in range(BLOC):
            nc.vector.tensor_scalar_mul(
                kd2[:, t, s, :], kb[:, t, :], d_c[:, t, s : s + 1]
            )
    # V = L diag(d) K for both samples: psum [128, s*256 | s*256+256] per pt
    p_vs = []
    for pt in range(2):
        p_v = pbig.tile([128, 512], F32, tag="big")
        for qt in range(2):
            nc.tensor.matmul(
                p_v, lb[:, qt, ts(pt)], kd2[:, qt, :, :],
                start=(qt == 0), stop=(qt == 1),
            )
        p_vs.append(p_v)
    for s in range(BLOC):
        # per-sample V in SBUF; vk = V_s .* K (gpsimd)
        v_sb = loop.tile([128, 2, H], BF16, tag="v_sb")
        vk = loop.tile([128, 2, H], BF16, tag="vk")
        for qt in range(2):
            nc.scalar.copy(out=v_sb[:, qt, :], in_=p_vs[qt][:, s * H : (s + 1) * H])
        for qt in range(2):
            nc.gpsimd.tensor_mul(vk[:, qt, :], v_sb[:, qt, :], kb[:, qt, :])
        # W = K diag(d) L ; U = K diag(d) V  (separate rhs, samples decoupled)
        qs, rs = [], []
        for pt in range(2):
            p_wu = pbig.tile([128, 512], F32, tag="big")
            for qt in range(2):
                nc.tensor.matmul(
                    p_wu[:, 0:H], kd2[:, qt, s, ts(pt)], lb[:, qt, :],
                    start=(qt == 0), stop=(qt == 1),
                )
            for qt in range(2):
                nc.tensor.matmul(
                    p_wu[:, H : 2 * H], kd2[:, qt, s, ts(pt)], v_sb[:, qt, :],
                    start=(qt == 0), stop=(qt == 1),
                )
            q_sb = loop.tile([128, H], BF16, tag="q_sb")
            nc.vector.tensor_mul(q_sb, p_wu[:, H : 2 * H], klb[:, pt, :])
            r_sb = loop.tile([128, H], BF16, tag="r_sb")
            nc.vector.tensor_mul(r_sb, vk[:, pt, :], p_wu[:, 0:H])
            qs.append(q_sb)
            rs.append(r_sb)
        # matvec F (h.*d) distributed over the 4 partial matrices
        p_f = psmall.tile([128, 2, 2], F32, tag="small")  # [mt, {e,hd}]
        for mt in range(2):
            idx = 0
            for qt in range(2):
                for mat in (qs[qt], rs[qt]):
                    nc.tensor.matmul(
                        p_f[:, mt, :], mat[:, ts(mt)], ehd_b[:, qt, s, :],
                        start=(idx == 0), stop=(idx == 3),
                    )
                    idx += 1
        scr = loop.tile([128, 2], F32, tag="scr")
        nc.vector.tensor_mul(scr, p_f[:, :, 1], hd_c[:, :, s])
        with nc.allow_low_precision("f32r accum rounding ~1e-6, fine here"):
            nc.vector.reduce_sum(acc_cols[:, s : s + 1], scr, axis=AX.X)

    # ---------------- final scalars & output ----------------
    p_sc = psmall.tile([BLOC, 2], F32, tag="small")
    nc.tensor.matmul(p_sc, acc_cols, ones2_r, start=True, stop=True)
    nc.scalar.mul(out=svals[:, 0:1], in_=p_sc[:, 0:1], mul=8.0)  # ||dG||^2

    # sqrt via bit trick + Newton (avoids 2nd ACT table load)
    y = work.tile([BLOC, 2], F32)
    nc.vector.tensor_scalar(
        out=y.bitcast(I32), in0=svals.bitcast(I32),
        scalar1=1, scalar2=None, op0=OP.arith_shift_right,
    )
    nc.vector.tensor_scalar(
        out=y.bitcast(I32), in0=y.bitcast(I32),
        scalar1=SQRT_MAGIC, scalar2=None, op0=OP.add,
    )
    rcp = work.tile([BLOC, 2], F32)
    qn = work.tile([BLOC, 2], F32)
    for _ in range(1):
        nc.vector.reciprocal(rcp, y)
        nc.vector.tensor_mul(qn, svals, rcp)          # s / y
        nc.vector.tensor_add(y, y, qn)                # y + s/y
        nc.vector.tensor_scalar_mul(y, y, 0.5)        # 0.5*(y + s/y)
    den = work.tile([BLOC, 1], F32)
    nc.vector.tensor_mul(den, y[:, 0:1], y[:, 1:2])
    inv = work.tile([BLOC, 1], F32)
    nc.vector.reciprocal(inv, den)

    # a = (0.5*S2 - S1) * inv ; out_bot = a + (-0.1*dev)
    s2h = work.tile([BLOC, N], F32)
    nc.scalar.copy(out=s2h, in_=p_s12[:, 128:256])
    comb = work.tile([BLOC, N], F32)
    nc.vector.tensor_sub(comb, s2h, p_s12[:, 0:128])
    bot = work.tile([BLOC, N], F32)
    nc.vector.scalar_tensor_tensor(
        out=bot, in0=comb, scalar=inv, in1=rest, op0=OP.mult, op1=OP.add
    )
    nc.sync.dma_start(out=d_out[:, :], in_=bot)


_NC_CACHE = None


def _get_nc():
    global _NC_CACHE
    if _NC_CACHE is None:
        _NC_CACHE = build_nc()
    return _NC_CACHE


def make_in_maps(inputs):
    """Shard full inputs into per-core input maps."""
    state = np.ascontiguousarray(np.asarray(inputs["state_batch"], dtype=np.float32))
    x0 = np.asarray(inputs["x0_batch"], dtype=np.float32)
    x1 = np.asarray(inputs["x1_batch"], dtype=np.float32)
    W1 = np.ascontiguousarray(np.asarray(inputs["W1"], dtype=np.float32))
    W2 = np.ascontiguousarray(np.asarray(inputs["W2"], dtype=np.float32))
    b1 = np.asarray(inputs["b1"], dtype=np.float32)
    t = np.float32(np.asarray(inputs["t"]).reshape(()))
    dev, vel = state[:B], state[B:]
    misc = np.zeros((128, 4), np.float32)
    misc[:, 0] = t
    misc[:, 1:3] = b1.reshape(2, 128).T
    misc = np.ascontiguousarray(misc)
    in_maps = []
    for c in range(NCORES):
        sl = slice(c * BLOC, (c + 1) * BLOC)
        vecs = np.concatenate([dev[sl], x0[sl], x1[sl], vel[sl]], axis=0)
        in_maps.append(
            {
                "vecs": np.ascontiguousarray(vecs),
                "W1": W1,
                "W2": W2,
                "misc": misc,
            }
        )
    return in_maps, vel


def kernel(**inputs) -> np.ndarray:
    from concourse.bass_utils import run_bass_kernel_spmd

    nc = _get_nc()
    in_maps, vel = make_in_maps(inputs)
    res = run_bass_kernel_spmd(nc, in_maps, core_ids=list(range(NCORES)))
    bottom = np.concatenate([res.results[c]["out_bot"] for c in range(NCORES)], axis=0)
    return np.concatenate([vel, bottom], axis=0).astype(np.float32)
